# revision 1
# baseline (speedup 1.0000x reference)
"""Trainium2 Bass kernel for the Luong-attention LSTM decoder (nn_Decoder).

8-core strategy:
- Host folds Wa into the recurrence: attn@Wk_a = h2@R' + ctx@C' with
  R' = Wa_top@Wk_a + lstm_r, C' = Wa_bot@Wk_a. The x-projection (+bias) is
  host-precomputed. attn2/logits defer to a batched epilogue.
- Recurrence replicated on all cores (weight-stream-bound); state kept
  transposed (h2T/ctxT) as matmul lhsT; gates pipelined in h-quarters.
- Attention batch-sharded (8 batches/core); one AllGather per step
  reassembles ctxT.
- Epilogue: attn2 = [h2,ctx]@Wa, logits = attn2@fc_w with vocab column-sharded
  4000/core; host concatenates vocab shards.
"""
import sys

sys.path.insert(0, "/opt/trn_rl_repo")

import numpy as np
import concourse.bass as bass
import concourse.tile as tile
from concourse import bacc, mybir
from concourse.bass_utils import run_bass_kernel_spmd

B, T_IN, T_DEC = 64, 64, 47
V, E, H = 32000, 512, 1024
NC = 8
BL = B // NC
VS = V // NC
KC = H // 128
NPAIR = BL // 2
GS = VS // 8
F32 = mybir.dt.float32
F16 = mybir.dt.float16
BF16 = mybir.dt.bfloat16

_CACHE = {}


def _build():
    nc = bacc.Bacc(None, target_bir_lowering=False)

    wz_d = nc.dram_tensor("wz", [16, 128, 4096], BF16, kind="ExternalInput")
    xp_d = nc.dram_tensor("xp", [T_DEC, B, 4096], BF16, kind="ExternalInput")
    h2t0_d = nc.dram_tensor("h2t0", [KC, 128, 64], BF16, kind="ExternalInput")
    c0_d = nc.dram_tensor("c0", [B, H], F32, kind="ExternalInput")
    wm_d = nc.dram_tensor("wm", [KC, 128, H], F32, kind="ExternalInput")
    memo_d = nc.dram_tensor("memo", [BL, T_IN, H], F16, kind="ExternalInput")
    memof_d = nc.dram_tensor("memof", [BL, T_IN, H], F32, kind="ExternalInput")
    idf_d = nc.dram_tensor("idf", [T_IN, T_IN], F32, kind="ExternalInput")
    isel_d = nc.dram_tensor("isel", [B, 72], F32, kind="ExternalInput")
    wa_d = nc.dram_tensor("wa", [16, 128, H], BF16, kind="ExternalInput")
    fcw_d = nc.dram_tensor("fcw", [KC, 128, VS], BF16, kind="ExternalInput")
    fcbr_d = nc.dram_tensor("fcbr", [128, VS], F32, kind="ExternalInput")
    idb_d = nc.dram_tensor("idb", [128, 128], BF16, kind="ExternalInput")
    out_d = nc.dram_tensor("out", [B, T_DEC, VS], F32, kind="ExternalOutput")

    h2h_d = nc.dram_tensor("h2hist", [T_DEC, 128, KC * B], BF16)
    ctxh_d = nc.dram_tensor("ctxhist", [T_DEC, 128, NC * KC * BL], BF16)
    agin = [nc.dram_tensor(f"agin{t}", [128, KC * BL], BF16) for t in range(T_DEC)]
    agout = [
        nc.dram_tensor(f"agout{t}", [NC * 128, KC * BL], BF16, addr_space="Shared")
        for t in range(T_DEC)
    ]
    rg = [list(range(NC))]

    with tile.TileContext(nc) as tc:
        with (
            tc.tile_pool(name="one", bufs=1) as one,
            tc.tile_pool(name="work", bufs=1) as work,
            tc.tile_pool(name="gat", bufs=2) as gat,
            tc.tile_pool(name="gz", bufs=1) as gz,
            tc.tile_pool(name="zps", bufs=2, space="PSUM") as zps,
            tc.tile_pool(name="tps", bufs=2, space="PSUM") as tps,
            tc.tile_pool(name="aps", bufs=2, space="PSUM") as aps,
        ):
            # ---------------- resident tiles ----------------
            # h2tx: [128, chunk, 72]: cols 0:64 full h2T, 64:72 own-batch h2T
            h2tx = one.tile([128, KC, 64], BF16, tag="h2tx")
            nc.gpsimd.dma_start(h2tx[:], h2t0_d.rearrange("c p b -> p c b")[:, :, 0:64])
            h2own = one.tile([128, KC, BL], F16, tag="h2own")
            nc.vector.memset(h2own[:], 0.0)
            isel = one.tile([B, 72], F32, tag="isel")
            nc.gpsimd.dma_start(isel[:], isel_d[:])
            ctxt = one.tile([128, KC, NC, BL], BF16, tag="ctxt")
            nc.vector.memset(ctxt[:], 0.0)
            cst = one.tile([B, H], F32, tag="cst")
            nc.gpsimd.dma_start(cst[:], c0_d[:])

            # keys for own batches: keysK[:, c2, b, t]
            keysK = one.tile([128, KC, BL, T_IN], F16)
            with (
                tc.tile_pool(name="boot", bufs=1) as boot,
                tc.tile_pool(name="bootw", bufs=2) as bootw,
                tc.tile_pool(name="bootm", bufs=1) as bootm,
            ):
                memT = boot.tile([128, KC, BL * T_IN], F32)
                idf = boot.tile([T_IN, T_IN], F32, tag="idf")
                nc.gpsimd.dma_start(idf[:], idf_d[:])
                for b in range(BL):
                    mrow = bootm.tile([T_IN, H], F32, tag="mrow")
                    nc.sync.dma_start(mrow[:], memof_d[b])
                    for c in range(KC):
                        mtp = tps.tile([128, T_IN], F32, tag="tp")
                        nc.tensor.transpose(
                            mtp[:, 0:T_IN], mrow[:, c * 128:(c + 1) * 128], idf[:]
                        )
                        nc.vector.tensor_copy(
                            memT[:, c, b * T_IN:(b + 1) * T_IN], mtp[:, 0:T_IN]
                        )
                for c2 in range(KC):
                    kp = aps.tile([128, BL * T_IN], F32, tag="a")
                    for k in range(KC):
                        wmc = bootw.tile([128, 128], F32, tag="wmc")
                        nc.gpsimd.dma_start(wmc[:], wm_d[k, :, c2 * 128:(c2 + 1) * 128])
                        nc.tensor.matmul(
                            kp[:], wmc[:], memT[:, k],
                            start=(k == 0), stop=(k == KC - 1),
                        )
                    nc.scalar.copy(keysK[:, c2], kp[:].rearrange("p (b t) -> p b t", b=BL))

            wz = one.tile([128, 16, 4096], BF16)
            for k in range(16):
                nc.sync.dma_start(wz[:, k], wz_d[k])

            # mem pair-packed for ctx: memPK[(parity*64+t), pair, chunk, h]
            memPK = one.tile([128, NPAIR, KC, 128], F16)
            for b in range(BL):
                half = (b % 2) * 64
                nc.sync.dma_start(
                    memPK[half:half + 64, b // 2].rearrange("t c h -> t (c h)"),
                    memo_d[b],
                )

            ones64 = one.tile([B, 1], F32, tag="ones64")
            nc.vector.memset(ones64[:], 1.0)
            onesr = one.tile([1, B], F32, tag="onesr")
            nc.vector.memset(onesr[:], 1.0)
            alignZ = one.tile([128, BL], F16, tag="alignZ")
            nc.vector.memset(alignZ[:], 0.0)

            # ================= decode loop =================
            for t in range(T_DEC):
                xpt = work.tile([B, 4096], BF16, tag="xp")
                nc.sync.dma_start(xpt[:], xp_d[t])
                h2b = work.tile([B, H], F32, tag="h2b")
                for qp in range(2):  # quarter-pairs; h2 chunks first so the
                    # AG-dependent ctx chunks overlap the collective
                    zq2 = [zps.tile([B, 1024], F32, tag="zq", name=f"zq{t}_{qp}_{j}") for j in range(2)]
                    for ks, ke in ((0, KC), (KC, 16)):
                        for j in range(2):
                            q = 2 * qp + j
                            for k in range(ks, ke):
                                if k < KC:
                                    lhs = h2tx[:, k]
                                else:
                                    lhs = ctxt[:, k - KC].rearrange("p r w -> p (r w)")
                                nc.tensor.matmul(
                                    zq2[j][:, 0:512], lhs,
                                    wz[:, k, q * 1024:q * 1024 + 512],
                                    start=(k == 0), stop=(k == 15),
                                )
                                nc.tensor.matmul(
                                    zq2[j][:, 512:1024], lhs,
                                    wz[:, k, q * 1024 + 512:(q + 1) * 1024],
                                    start=(k == 0), stop=(k == 15),
                                )
                    for j in range(2):
                        q = 2 * qp + j
                        zq = zq2[j]
                    # z2 = z + xproj ; gate order within zq: i,f,g,o
                        z2 = gz.tile([B, 4, 256], F32, tag="z2")
                        nc.vector.scalar_tensor_tensor(
                            z2[:], zq[:].rearrange("b (g n) -> b g n", g=4),
                            1.0, xpt[:, q * 1024:(q + 1) * 1024].rearrange("b (g n) -> b g n", g=4),
                            mybir.AluOpType.mult, mybir.AluOpType.add,
                        )
                        sif = gat.tile([B, 512], F32, tag="sif")
                        nc.scalar.activation(
                            sif[:].rearrange("b (a n) -> b a n", a=2), z2[:, 0:2],
                            mybir.ActivationFunctionType.Sigmoid)
                        so = gat.tile([B, 256], F32, tag="so")
                        nc.scalar.activation(so[:], z2[:, 3],
                                             mybir.ActivationFunctionType.Sigmoid)
                        tg = gat.tile([B, 256], F32, tag="tg")
                        nc.scalar.activation(tg[:], z2[:, 2],
                                             mybir.ActivationFunctionType.Tanh)
                        qs = slice(q * 256, (q + 1) * 256)
                        nc.vector.tensor_mul(sif[:, 256:512], sif[:, 256:512], cst[:, qs])
                        nc.vector.tensor_mul(sif[:, 0:256], sif[:, 0:256], tg[:])
                        nc.vector.tensor_add(cst[:, qs], sif[:, 256:512], sif[:, 0:256])
                        nc.scalar.activation(tg[:], cst[:, qs],
                                             mybir.ActivationFunctionType.Tanh)
                        nc.vector.tensor_mul(h2b[:, qs], so[:], tg[:])

                # transpose h2 (+ own-col gather): [64,128] @ [64,72]
                for c in range(KC):
                    tp = tps.tile([128, 72], F32, tag="tp")
                    nc.tensor.matmul(tp[:], h2b[:, c * 128:(c + 1) * 128], isel[:],
                                     start=True, stop=True)
                    nc.scalar.copy(h2tx[:, c, :], tp[:, 0:64])
                    nc.vector.tensor_copy(h2own[:, c, :], tp[:, 64:72])
                nc.sync.dma_start(
                    h2h_d[t].rearrange("p (c b) -> p c b", c=KC), h2tx[:]
                )

                # ---- score (own batches): scT8[t, j] ----
                scT8 = aps.tile([64, BL], F32, tag="a")
                for j in range(BL):
                    for c in range(KC):
                        nc.tensor.matmul(
                            scT8[:, j:j + 1], keysK[:, c, j, :],
                            h2own[:, c, j:j + 1],
                            start=(c == 0), stop=(c == KC - 1),
                        )
                e8 = gat.tile([64, BL], F32, tag="e8")
                nc.scalar.activation(e8[:], scT8[:], mybir.ActivationFunctionType.Exp)
                s18 = tps.tile([1, BL], F32, tag="tp")
                nc.tensor.matmul(s18[:], ones64[:], e8[:], start=True, stop=True)
                r18 = gat.tile([1, BL], F32, tag="r18")
                nc.vector.reciprocal(r18[:], s18[:])
                rb = tps.tile([64, BL], F32, tag="tp")
                nc.tensor.matmul(rb[:], onesr[:], r18[:], start=True, stop=True)
                a8 = gat.tile([64, BL], F16, tag="a8")
                nc.vector.tensor_mul(a8[:], e8[:], rb[:])
                # scatter: even own-batches -> upper half, odd -> lower half
                nc.vector.tensor_copy(
                    alignZ[0:64, :].rearrange("p (pr two) -> p pr two", two=2)[:, :, 0],
                    a8[:].rearrange("p (pr two) -> p pr two", two=2)[:, :, 0],
                )
                nc.vector.tensor_copy(
                    alignZ[64:128, :].rearrange("p (pr two) -> p pr two", two=2)[:, :, 1],
                    a8[:].rearrange("p (pr two) -> p pr two", two=2)[:, :, 1],
                )

                # ---- ctx (own batches, pair-packed block-diag) ----
                ctxPS = aps.tile([128, KC, BL], F32, tag="a")
                for pr in range(NPAIR):
                    for c in range(KC):
                        nc.tensor.matmul(
                            ctxPS[:, c, 2 * pr:2 * pr + 2],
                            memPK[:, pr, c, :],
                            alignZ[:, 2 * pr:2 * pr + 2],
                            start=True, stop=True,
                        )
                ctxo = gat.tile([128, KC, BL], BF16, tag="ctxo")
                nc.scalar.copy(ctxo[:], ctxPS[:])

                # ---- AllGather ctx ----
                nc.gpsimd.dma_start(agin[t][:], ctxo[:].rearrange("p c w -> p (c w)"))
                nc.gpsimd.collective_compute(
                    "AllGather", mybir.AluOpType.bypass,
                    replica_groups=rg,
                    ins=[agin[t][:]], outs=[agout[t][:]],
                )
                for r in range(NC):
                    nc.gpsimd.dma_start(
                        ctxt[:, :, r, :],
                        agout[t][r * 128:(r + 1) * 128, :].rearrange("p (c w) -> p c w", c=KC),
                    )
                nc.sync.dma_start(
                    ctxh_d[t], ctxt[:].rearrange("p c r w -> p (c r w)")
                )

        # ================= epilogue =================
        with (
            tc.tile_pool(name="eone", bufs=1) as eone,
            tc.tile_pool(name="ework", bufs=3) as ework,
            tc.tile_pool(name="eps", bufs=2, space="PSUM") as eps,
            tc.tile_pool(name="fps", bufs=2, space="PSUM") as fps,
        ):
            wa_sb = eone.tile([128, 16, H], BF16)
            nc.sync.dma_start(wa_sb[:], wa_d.rearrange("k p h -> p k h"))
            fcw_sb = eone.tile([128, KC, VS], BF16)
            nc.sync.dma_start(fcw_sb[:], fcw_d.rearrange("k p v -> p k v"))
            identb = eone.tile([128, 128], BF16)
            nc.gpsimd.dma_start(identb[:], idb_d[:])
            fcbR = eone.tile([128, VS], F32)
            nc.gpsimd.dma_start(fcbR[:], fcbr_d[:])

            for p in range((T_DEC + 1) // 2):
                t0 = 2 * p
                nsteps = 2 if t0 + 1 < T_DEC else 1
                M = 64 * nsteps
                h2p = ework.tile([128, KC, 2, B], BF16, tag="h2p")
                ctxp = ework.tile([128, KC, 2, B], BF16, tag="ctxp")
                for i in range(nsteps):
                    nc.sync.dma_start(
                        h2p[:, :, i, :],
                        h2h_d[t0 + i].rearrange("p (c b) -> p c b", c=KC),
                    )
                    nc.sync.dma_start(
                        ctxp[:, :, i, :],
                        ctxh_d[t0 + i].rearrange("p (c b) -> p c b", c=KC),
                    )
                a2 = eps.tile([128, H], F32, tag="a2")
                for k in range(16):
                    if k < KC:
                        lhs = h2p[:, k, 0:nsteps, :].rearrange("p s b -> p (s b)")
                    else:
                        lhs = ctxp[:, k - KC, 0:nsteps, :].rearrange("p s b -> p (s b)")
                    for n in range(2):
                        nc.tensor.matmul(
                            a2[0:M, n * 512:(n + 1) * 512],
                            lhs, wa_sb[:, k, n * 512:(n + 1) * 512],
                            start=(k == 0), stop=(k == 15),
                        )
                a2sb = ework.tile([128, H], BF16, tag="a2sb")
                nc.scalar.copy(a2sb[0:M, :], a2[0:M, :])
                a2t = ework.tile([128, KC, 128], BF16, tag="a2t")
                for c2 in range(KC):
                    tp = eps.tile([128, 128], BF16, tag="a2tp")
                    nc.tensor.transpose(
                        tp[:, 0:M], a2sb[0:M, c2 * 128:(c2 + 1) * 128], identb[0:M, 0:M]
                    )
                    nc.scalar.copy(a2t[:, c2, 0:M], tp[:, 0:M])
                for g in range(8):
                    lg = fps.tile([128, GS], F32, tag="lg")
                    for k in range(KC):
                        nc.tensor.matmul(
                            lg[0:M, :], a2t[:, k, 0:M],
                            fcw_sb[:, k, g * GS:(g + 1) * GS],
                            start=(k == 0), stop=(k == KC - 1),
                        )
                    lgs = ework.tile([128, GS], F32, tag="lgs")
                    nc.vector.scalar_tensor_tensor(
                        lgs[0:M, :], lg[0:M, :], 1.0, fcbR[0:M, g * GS:(g + 1) * GS],
                        mybir.AluOpType.mult, mybir.AluOpType.add,
                    )
                    for i in range(nsteps):
                        nc.sync.dma_start(
                            out_d[:, t0 + i, g * GS:(g + 1) * GS],
                            lgs[i * 64:(i + 1) * 64, :],
                        )

    nc.finalize()
    return nc


def _prep_inputs(inputs):
    bfnp = mybir.dt.np(BF16)
    f32 = lambda x: np.asarray(x, dtype=np.float32)
    tokens = np.asarray(inputs["tokens"])
    memory = f32(inputs["memory"])
    enc_h = f32(inputs["enc_h"])
    enc_c = f32(inputs["enc_c"])
    emb = f32(inputs["emb"])
    Wm = f32(inputs["Wm"])
    Wa = f32(inputs["Wa"])
    lstm_k = f32(inputs["lstm_k"])
    lstm_r = f32(inputs["lstm_r"])
    lstm_b = f32(inputs["lstm_b"])
    fc_w = f32(inputs["fc_w"])
    fc_b = f32(inputs["fc_b"])

    Wk_x = lstm_k[:E]
    Wk_a = lstm_k[E:]
    Rp = Wa[:H] @ Wk_a + lstm_r
    Cp = Wa[H:] @ Wk_a
    wzf = np.concatenate([Rp, Cp], 0)
    wzf = wzf.reshape(2048, 4, 4, 256).transpose(0, 2, 1, 3).reshape(2048, 4096)
    wz = np.ascontiguousarray(wzf).reshape(16, 128, 4096).astype(bfnp)
    xs = emb[tokens]                                   # [B, T_DEC, E]
    xpb = xs @ Wk_x + lstm_b
    # t=0 folding correction: attn_0 = 0 (not [enc_h,0]@Wa) and h_0 = enc_h;
    # absorb enc_h@lstm_r into xproj[0] and start the device h2 state at zero.
    xpb[:, 0] += enc_h @ lstm_r
    xpf = xpb.transpose(1, 0, 2)
    xpf = xpf.reshape(T_DEC, B, 4, 4, 256).transpose(0, 1, 3, 2, 4)
    xp = np.ascontiguousarray(xpf).reshape(T_DEC, B, 4096).astype(bfnp)
    wm = np.ascontiguousarray(Wm.reshape(KC, 128, H), np.float32)
    wa = Wa.reshape(16, 128, H).astype(bfnp)
    idb = np.eye(128, dtype=np.float32).astype(bfnp)

    common = dict(wz=wz, xp=xp, wm=wm, wa=wa, c0=enc_c.copy(), idb=idb,
                  idf=np.eye(T_IN, dtype=np.float32))
    h2t_full = np.zeros((KC, 128, B), np.float32)
    maps = []
    for r in range(NC):
        own = slice(r * BL, (r + 1) * BL)
        sel = np.zeros((B, BL), np.float32)
        sel[np.arange(r * BL, (r + 1) * BL), np.arange(BL)] = 1.0
        isel = np.concatenate([np.eye(B, dtype=np.float32), sel], axis=1)
        h2t0 = h2t_full
        maps.append(dict(
            common,
            h2t0=np.ascontiguousarray(h2t0).astype(bfnp),
            memo=memory[own].astype(np.float16),
            memof=np.ascontiguousarray(memory[own], np.float32),
            isel=np.ascontiguousarray(isel),
            fcw=np.ascontiguousarray(
                fc_w[:, r * VS:(r + 1) * VS]).reshape(KC, 128, VS).astype(bfnp),
            fcbr=np.ascontiguousarray(
                np.broadcast_to(fc_b[r * VS:(r + 1) * VS], (128, VS)), np.float32),
        ))
    return maps


def kernel(**inputs):
    if "nc" not in _CACHE:
        _CACHE["nc"] = _build()
    nc = _CACHE["nc"]
    maps = _prep_inputs(inputs)
    res = run_bass_kernel_spmd(nc, maps, list(range(NC)))
    global LAST_RESULT
    LAST_RESULT = res
    out = np.concatenate([res.results[r]["out"] for r in range(NC)], axis=2)
    return out


LAST_RESULT = None



# revision 5
# speedup vs baseline: 1.1771x; 1.1771x over previous
"""Trainium2 Bass kernel for the Luong-attention LSTM decoder (nn_Decoder).

8-core strategy (v1, sharded recurrence):
- Host folds Wa into the recurrence: z = [H*, ctx] @ wz + xp with
  wz = [0.5*(Wa_top@Wk_a + lstm_r); Wa_bot@Wk_a], xp precomputed (+bias,
  t=0 correction). State H* = 2*h2 (tanh-only gates); consumers pre-scaled.
- z matmul + gates column-sharded 8-way: core r computes gate columns
  [i_r|f_r|o_r|g_r] (128 each), keeps its c-state slice. AllGather #1
  reassembles full transposed H*; AllGather #2 reassembles ctx.
- Attention batch-sharded (8 samples/core): cross-scores via 8 matmuls +
  matmul-based diagonal select (SPMD-uniform via per-core one-hot isel).
- Epilogue: attn2 = [H*,ctx]@wa_eff, logits vocab-sharded 4000/core; host
  concatenates vocab shards.
"""
import sys

sys.path.insert(0, "/opt/trn_rl_repo")

import numpy as np
import concourse.bass as bass
import concourse.tile as tile
from concourse import bacc, mybir
from concourse.bass_utils import run_bass_kernel_spmd

B, T_IN, T_DEC = 64, 64, 47
V, E, H = 32000, 512, 1024
NC = 8
BL = B // NC
VS = V // NC
KC = H // 128
NPAIR = BL // 2
GS = VS // 8
F32 = mybir.dt.float32
F16 = mybir.dt.float16
BF16 = mybir.dt.bfloat16
TANH = mybir.ActivationFunctionType.Tanh
EXP = mybir.ActivationFunctionType.Exp
MULT = mybir.AluOpType.mult
ADD = mybir.AluOpType.add

_CACHE = {}


def _build():
    nc = bacc.Bacc(None, target_bir_lowering=False)

    wz_d = nc.dram_tensor("wz", [16, 128, 512], BF16, kind="ExternalInput")
    xp_d = nc.dram_tensor("xp", [T_DEC, B, 512], BF16, kind="ExternalInput")
    c0_d = nc.dram_tensor("c0", [B, 128], F32, kind="ExternalInput")
    keysk_d = nc.dram_tensor("keysk", [128, KC, BL, T_IN], BF16, kind="ExternalInput")
    mempk_d = nc.dram_tensor("mempk", [128, NPAIR, KC, 128], F16, kind="ExternalInput")
    isel_d = nc.dram_tensor("isel", [B, BL], F32, kind="ExternalInput")
    idf_d = nc.dram_tensor("idf", [64, 64], F32, kind="ExternalInput")
    wa_d = nc.dram_tensor("wa", [16, 128, H], BF16, kind="ExternalInput")
    fcw_d = nc.dram_tensor("fcw", [KC, 128, VS], BF16, kind="ExternalInput")
    fcbr_d = nc.dram_tensor("fcbr", [128, VS], F32, kind="ExternalInput")
    idb_d = nc.dram_tensor("idb", [128, 128], BF16, kind="ExternalInput")
    out_d = nc.dram_tensor("out", [B, T_DEC, VS], F32, kind="ExternalOutput")

    h2h_d = nc.dram_tensor("h2hist", [T_DEC, 128, KC * B], BF16)
    ctxh_d = nc.dram_tensor("ctxhist", [T_DEC, 128, KC * B], BF16)
    ag1i = [nc.dram_tensor(f"ag1i{t}", [128, 64], BF16) for t in range(T_DEC)]
    ag1o = [
        nc.dram_tensor(f"ag1o{t}", [NC * 128, 64], BF16, addr_space="Shared")
        for t in range(T_DEC)
    ]
    ag2i = [nc.dram_tensor(f"ag2i{t}", [128, 64], BF16) for t in range(T_DEC)]
    ag2o = [
        nc.dram_tensor(f"ag2o{t}", [NC * 128, 64], BF16, addr_space="Shared")
        for t in range(T_DEC)
    ]
    rg = [list(range(NC))]

    with tile.TileContext(nc) as tc:
        with (
            tc.tile_pool(name="one", bufs=1) as one,
            tc.tile_pool(name="work", bufs=2) as work,
            tc.tile_pool(name="gat", bufs=2) as gat,
            tc.tile_pool(name="zps", bufs=2, space="PSUM") as zps,
            tc.tile_pool(name="tps", bufs=2, space="PSUM") as tps,
            tc.tile_pool(name="aps", bufs=1, space="PSUM") as aps,
            tc.tile_pool(name="sps", bufs=1, space="PSUM") as sps,
        ):
            # ---------------- resident tiles ----------------
            wz = one.tile([128, 16, 512], BF16)
            nc.sync.dma_start(wz[:], wz_d.rearrange("k p n -> p k n"))
            keysK = one.tile([128, KC, BL, T_IN], BF16)
            nc.sync.dma_start(keysK[:], keysk_d[:])
            memPK = one.tile([128, NPAIR, KC, 128], F16)
            nc.sync.dma_start(memPK[:], mempk_d[:])
            isel = one.tile([B, BL], F32, tag="isel")
            nc.gpsimd.dma_start(isel[:], isel_d[:])
            idf = one.tile([64, 64], F32, tag="idf")
            nc.gpsimd.dma_start(idf[:], idf_d[:])
            cst = one.tile([B, 128], F32, tag="cst")
            nc.gpsimd.dma_start(cst[:], c0_d[:])

            h2T = one.tile([128, KC, 64], BF16, tag="h2T")
            nc.vector.memset(h2T[:], 0.0)
            ctxT = one.tile([128, KC, 64], BF16, tag="ctxT")
            nc.vector.memset(ctxT[:], 0.0)
            alignZ = one.tile([128, BL], F16, tag="alignZ")
            nc.vector.memset(alignZ[:], 0.0)
            ones64 = one.tile([B, 1], F32, tag="ones64")
            nc.vector.memset(ones64[:], 1.0)
            onesr = one.tile([1, B], F32, tag="onesr")
            nc.vector.memset(onesr[:], 1.0)

            # ================= decode loop =================
            for t in range(T_DEC):
                xpt = work.tile([B, 512], BF16, tag="xp")
                nc.sync.dma_start(xpt[:], xp_d[t])

                # z = [H*,ctx] @ wz  (k 0..7: H* tiles; 8..15: ctx tiles)
                zq = zps.tile([B, 512], F32, tag="zq")
                for k in range(16):
                    lhs = h2T[:, k] if k < KC else ctxT[:, k - KC]
                    nc.tensor.matmul(
                        zq[:], lhs, wz[:, k],
                        start=(k == 0), stop=(k == 15),
                    )

                # gates: layout [i|f|o|g] x128; H* = 2h, D = 2c
                z2 = gat.tile([B, 512], F32, tag="z2")
                nc.vector.scalar_tensor_tensor(z2[:], zq[:], 1.0, xpt[:], MULT, ADD)
                tio = gat.tile([B, 384], F32, tag="tio")
                nc.scalar.activation(tio[:], z2[:, 0:384], TANH, scale=0.5)
                tg = gat.tile([B, 128], F32, tag="tg")
                nc.scalar.activation(tg[:], z2[:, 384:512], TANH)
                av = gat.tile([B, 128], F32, tag="av")
                nc.vector.scalar_tensor_tensor(av[:], tio[:, 128:256], 1.0, cst[:], ADD, MULT)
                bv = gat.tile([B, 128], F32, tag="bv")
                nc.vector.scalar_tensor_tensor(bv[:], tio[:, 0:128], 1.0, tg[:], ADD, MULT)
                nc.vector.scalar_tensor_tensor(cst[:], av[:], 0.5, bv[:], MULT, ADD)
                tc2 = gat.tile([B, 128], F32, tag="tc2")
                nc.scalar.activation(tc2[:], cst[:], TANH, scale=0.5)
                h2s = gat.tile([B, 128], F32, tag="h2s")
                nc.vector.scalar_tensor_tensor(h2s[:], tio[:, 256:384], 1.0, tc2[:], ADD, MULT)

                # transpose own H* slice -> [128, 64] bf16, stage + AG1
                tp = tps.tile([128, 64], F32, tag="tp")
                nc.tensor.transpose(tp[:], h2s[:], idf[:])
                h2sT = work.tile([128, 64], BF16, tag="h2sT")
                nc.scalar.copy(h2sT[:], tp[:])
                nc.sync.dma_start(ag1i[t][:], h2sT[:])
                nc.gpsimd.collective_compute(
                    "AllGather", mybir.AluOpType.bypass,
                    replica_groups=rg,
                    ins=[ag1i[t][:]], outs=[ag1o[t][:]],
                )
                nc.sync.dma_start(
                    h2T[:], ag1o[t].rearrange("(q p) b -> p q b", q=NC)
                )
                nc.sync.dma_start(
                    h2h_d[t], h2T[:].rearrange("p c b -> p (c b)")
                )

                # ---- scores for own 8 samples via cross + diag-select ----
                crossP = aps.tile([64, 512], F32, tag="crossP")
                for c in range(KC):
                    nc.tensor.matmul(
                        crossP[:], h2T[:, c],
                        keysK[:, c].rearrange("p j t -> p (j t)"),
                        start=(c == 0), stop=(c == KC - 1),
                    )
                ecr = gat.tile([64, 512], F32, tag="ecr")
                nc.scalar.activation(ecr[:], crossP[:], EXP)
                smx = sps.tile([64, 3 * BL], F32, tag="smx")
                for j in range(BL):
                    nc.tensor.matmul(
                        smx[:, j:j + 1], ecr[:, j * 64:(j + 1) * 64],
                        isel[:, j:j + 1], start=True, stop=True,
                    )
                e8 = gat.tile([64, BL], F32, tag="e8")
                nc.vector.tensor_copy(e8[:], smx[:, 0:BL])
                nc.tensor.matmul(smx[0:1, BL:2 * BL], ones64[:], e8[:],
                                 start=True, stop=True)
                r18 = gat.tile([1, BL], F32, tag="r18")
                nc.vector.reciprocal(r18[:], smx[0:1, BL:2 * BL])
                nc.tensor.matmul(smx[:, 2 * BL:3 * BL], onesr[:], r18[:],
                                 start=True, stop=True)
                a8 = gat.tile([64, BL], F16, tag="a8")
                nc.vector.tensor_mul(a8[:], e8[:], smx[:, 2 * BL:3 * BL])
                # scatter: even own-samples -> upper half, odd -> lower half
                nc.vector.tensor_copy(
                    alignZ[0:64, :].rearrange("p (pr two) -> p pr two", two=2)[:, :, 0],
                    a8[:].rearrange("p (pr two) -> p pr two", two=2)[:, :, 0],
                )
                nc.vector.tensor_copy(
                    alignZ[64:128, :].rearrange("p (pr two) -> p pr two", two=2)[:, :, 1],
                    a8[:].rearrange("p (pr two) -> p pr two", two=2)[:, :, 1],
                )

                # ---- ctx (own batches, pair-packed block-diag) ----
                ctxPS = aps.tile([128, KC, BL], F32, tag="ctxPS")
                for pr in range(NPAIR):
                    for c in range(KC):
                        nc.tensor.matmul(
                            ctxPS[:, c, 2 * pr:2 * pr + 2],
                            memPK[:, pr, c, :],
                            alignZ[:, 2 * pr:2 * pr + 2],
                            start=True, stop=True,
                        )
                ctxo = work.tile([128, KC, BL], BF16, tag="ctxo")
                nc.scalar.copy(ctxo[:], ctxPS[:])
                nc.sync.dma_start(ag2i[t][:], ctxo[:].rearrange("p c j -> p (c j)"))
                nc.gpsimd.collective_compute(
                    "AllGather", mybir.AluOpType.bypass,
                    replica_groups=rg,
                    ins=[ag2i[t][:]], outs=[ag2o[t][:]],
                )
                for q in range(NC):
                    nc.sync.dma_start(
                        ctxT[:, :, q * BL:(q + 1) * BL],
                        ag2o[t][q * 128:(q + 1) * 128, :].rearrange(
                            "p (c j) -> p c j", c=KC),
                    )
                nc.sync.dma_start(
                    ctxh_d[t], ctxT[:].rearrange("p c b -> p (c b)")
                )

        # ================= epilogue =================
        with (
            tc.tile_pool(name="eone", bufs=1) as eone,
            tc.tile_pool(name="ework", bufs=3) as ework,
            tc.tile_pool(name="eps", bufs=2, space="PSUM") as eps,
            tc.tile_pool(name="fps", bufs=2, space="PSUM") as fps,
        ):
            wa_sb = eone.tile([128, 16, H], BF16)
            nc.sync.dma_start(wa_sb[:], wa_d.rearrange("k p h -> p k h"))
            fcw_sb = eone.tile([128, KC, VS], BF16)
            nc.sync.dma_start(fcw_sb[:], fcw_d.rearrange("k p v -> p k v"))
            identb = eone.tile([128, 128], BF16)
            nc.gpsimd.dma_start(identb[:], idb_d[:])
            fcbR = eone.tile([128, VS], F32)
            nc.gpsimd.dma_start(fcbR[:], fcbr_d[:])

            for p in range((T_DEC + 1) // 2):
                t0 = 2 * p
                nsteps = 2 if t0 + 1 < T_DEC else 1
                M = 64 * nsteps
                h2p = ework.tile([128, KC, 2, B], BF16, tag="h2p")
                ctxp = ework.tile([128, KC, 2, B], BF16, tag="ctxp")
                for i in range(nsteps):
                    nc.sync.dma_start(
                        h2p[:, :, i, :],
                        h2h_d[t0 + i].rearrange("p (c b) -> p c b", c=KC),
                    )
                    nc.sync.dma_start(
                        ctxp[:, :, i, :],
                        ctxh_d[t0 + i].rearrange("p (c b) -> p c b", c=KC),
                    )
                a2 = eps.tile([128, H], F32, tag="a2")
                for k in range(16):
                    if k < KC:
                        lhs = h2p[:, k, 0:nsteps, :].rearrange("p s b -> p (s b)")
                    else:
                        lhs = ctxp[:, k - KC, 0:nsteps, :].rearrange("p s b -> p (s b)")
                    for n in range(2):
                        nc.tensor.matmul(
                            a2[0:M, n * 512:(n + 1) * 512],
                            lhs, wa_sb[:, k, n * 512:(n + 1) * 512],
                            start=(k == 0), stop=(k == 15),
                        )
                a2sb = ework.tile([128, H], BF16, tag="a2sb")
                nc.scalar.copy(a2sb[0:M, :], a2[0:M, :])
                a2t = ework.tile([128, KC, 128], BF16, tag="a2t")
                for c2 in range(KC):
                    tpe = eps.tile([128, 128], BF16, tag="a2tp")
                    nc.tensor.transpose(
                        tpe[:, 0:M], a2sb[0:M, c2 * 128:(c2 + 1) * 128], identb[0:M, 0:M]
                    )
                    nc.scalar.copy(a2t[:, c2, 0:M], tpe[:, 0:M])
                for g in range(8):
                    lg = fps.tile([128, GS], F32, tag="lg")
                    for k in range(KC):
                        nc.tensor.matmul(
                            lg[0:M, :], a2t[:, k, 0:M],
                            fcw_sb[:, k, g * GS:(g + 1) * GS],
                            start=(k == 0), stop=(k == KC - 1),
                        )
                    lgs = ework.tile([128, GS], F32, tag="lgs")
                    nc.vector.scalar_tensor_tensor(
                        lgs[0:M, :], lg[0:M, :], 1.0, fcbR[0:M, g * GS:(g + 1) * GS],
                        MULT, ADD,
                    )
                    for i in range(nsteps):
                        nc.sync.dma_start(
                            out_d[:, t0 + i, g * GS:(g + 1) * GS],
                            lgs[i * 64:(i + 1) * 64, :],
                        )

    nc.finalize()
    return nc


def _prep_inputs(inputs):
    bfnp = mybir.dt.np(BF16)
    f32 = lambda x: np.asarray(x, dtype=np.float32)
    tokens = np.asarray(inputs["tokens"])
    memory = f32(inputs["memory"])
    enc_h = f32(inputs["enc_h"])
    enc_c = f32(inputs["enc_c"])
    emb = f32(inputs["emb"])
    Wm = f32(inputs["Wm"])
    Wa = f32(inputs["Wa"])
    lstm_k = f32(inputs["lstm_k"])
    lstm_r = f32(inputs["lstm_r"])
    lstm_b = f32(inputs["lstm_b"])
    fc_w = f32(inputs["fc_w"])
    fc_b = f32(inputs["fc_b"])

    Wk_x = lstm_k[:E]
    Wk_a = lstm_k[E:]
    Rp = Wa[:H] @ Wk_a + lstm_r
    Cp = Wa[H:] @ Wk_a
    wz_full = np.concatenate([0.5 * Rp, Cp], axis=0)       # [2048, 4096]
    xs = emb[tokens]                                        # [B, T, E]
    xpb = (xs @ Wk_x + lstm_b).transpose(1, 0, 2).copy()    # [T, B, 4096]
    xpb[0] += enc_h @ lstm_r                                # t=0 folding fix
    keys = memory @ (0.5 * Wm)                              # [B, T_in, H]
    wa_eff = np.concatenate([0.5 * Wa[:H], Wa[H:]], axis=0)

    common = dict(
        idf=np.eye(64, dtype=np.float32),
        idb=np.eye(128, dtype=np.float32).astype(bfnp),
        wa=np.ascontiguousarray(wa_eff.reshape(16, 128, H)).astype(bfnp),
    )
    maps = []
    for r in range(NC):
        cols = np.concatenate([
            np.arange(r * 128, r * 128 + 128),
            H + np.arange(r * 128, r * 128 + 128),
            3 * H + np.arange(r * 128, r * 128 + 128),
            2 * H + np.arange(r * 128, r * 128 + 128),
        ])
        wz_r = np.ascontiguousarray(wz_full[:, cols]).reshape(16, 128, 512)
        xp_r = np.ascontiguousarray(xpb[:, :, cols])
        own = slice(r * BL, (r + 1) * BL)
        kk = keys[own].transpose(2, 0, 1)                   # [H, 8, T_in]
        keysK_r = np.ascontiguousarray(
            kk.reshape(KC, 128, BL, T_IN).transpose(1, 0, 2, 3))
        m = memory[own]                                     # [8, T_in, H]
        memPK_r = np.zeros((128, NPAIR, KC, 128), np.float16)
        for j in range(BL):
            memPK_r[(j % 2) * 64:(j % 2) * 64 + 64, j // 2] = (
                m[j].reshape(T_IN, KC, 128))
        isel_r = np.zeros((B, BL), np.float32)
        isel_r[r * BL + np.arange(BL), np.arange(BL)] = 1.0
        maps.append(dict(
            common,
            wz=wz_r.astype(bfnp),
            xp=xp_r.astype(bfnp),
            c0=np.ascontiguousarray(2.0 * enc_c[:, r * 128:(r + 1) * 128]),
            keysk=keysK_r.astype(bfnp),
            mempk=memPK_r,
            isel=isel_r,
            fcw=np.ascontiguousarray(
                fc_w[:, r * VS:(r + 1) * VS]).reshape(KC, 128, VS).astype(bfnp),
            fcbr=np.ascontiguousarray(
                np.broadcast_to(fc_b[r * VS:(r + 1) * VS], (128, VS)), np.float32),
        ))
    return maps


def kernel(**inputs):
    if "nc" not in _CACHE:
        _CACHE["nc"] = _build()
    nc = _CACHE["nc"]
    maps = _prep_inputs(inputs)
    res = run_bass_kernel_spmd(nc, maps, list(range(NC)))
    global LAST_RESULT
    LAST_RESULT = res
    out = np.concatenate([res.results[r]["out"] for r in range(NC)], axis=2)
    return out


LAST_RESULT = None


# revision 7
# speedup vs baseline: 1.3511x; 1.1478x over previous
"""Trainium2 Bass kernel for the Luong-attention LSTM decoder (nn_Decoder).

8-core strategy (v2, sharded recurrence + interleaved epilogue):
- Host folds Wa into the recurrence: z = [H*, ctx] @ wz + xp with
  wz = [0.5*(Wa_top@Wk_a + lstm_r); Wa_bot@Wk_a], xp precomputed (+bias,
  t=0 correction). State H* = 2*h2 (tanh-only gates, no ACT table
  switches); consumers pre-scaled by 0.5 on host.
- z matmul + gates column-sharded 8-way: core r computes gate columns
  [i_r|f_r|o_r|g_r] (128 each), keeps its c-state slice. AllGather #1
  reassembles transposed H*; AllGather #2 reassembles ctx. Single-DMA
  gathers.
- Attention batch-sharded (8 samples/core): cross-scores via 8 matmuls +
  matmul diagonal-select (SPMD-uniform via per-core one-hot isel).
- Epilogue (attn2 = [H*,ctx]@wa_eff, logits vocab-sharded 4000/core) is
  chunked and pumped into the AG-wait windows of later steps, keeping the
  PE HAM-warm and removing the serial tail. Pair history kept in SBUF.
"""
import sys

sys.path.insert(0, "/opt/trn_rl_repo")

from collections import deque

import numpy as np
import concourse.bass as bass
import concourse.tile as tile
from concourse import bacc, mybir
from concourse.bass_utils import run_bass_kernel_spmd

B, T_IN, T_DEC = 64, 64, 47
V, E, H = 32000, 512, 1024
NC = 8
BL = B // NC
VS = V // NC
KC = H // 128
NPAIR = BL // 2
GS = VS // 8
NPAIRS_T = (T_DEC + 1) // 2
F32 = mybir.dt.float32
F16 = mybir.dt.float16
BF16 = mybir.dt.bfloat16
TANH = mybir.ActivationFunctionType.Tanh
EXP = mybir.ActivationFunctionType.Exp
MULT = mybir.AluOpType.mult
ADD = mybir.AluOpType.add

_CACHE = {}


def _build():
    nc = bacc.Bacc(None, target_bir_lowering=False)

    wz_d = nc.dram_tensor("wz", [16, 128, 512], BF16, kind="ExternalInput")
    xp_d = nc.dram_tensor("xp", [T_DEC, B, 512], BF16, kind="ExternalInput")
    c0_d = nc.dram_tensor("c0", [B, 128], F32, kind="ExternalInput")
    keysk_d = nc.dram_tensor("keysk", [128, KC, BL, T_IN], BF16, kind="ExternalInput")
    mempk_d = nc.dram_tensor("mempk", [128, NPAIR, KC, 128], F16, kind="ExternalInput")
    isel_d = nc.dram_tensor("isel", [B, BL], F32, kind="ExternalInput")
    idf_d = nc.dram_tensor("idf", [64, 64], F32, kind="ExternalInput")
    wa_d = nc.dram_tensor("wa", [16, 128, H], BF16, kind="ExternalInput")
    fcw_d = nc.dram_tensor("fcw", [KC, 128, VS], BF16, kind="ExternalInput")
    fcbr_d = nc.dram_tensor("fcbr", [128, VS], F32, kind="ExternalInput")
    idb_d = nc.dram_tensor("idb", [128, 128], BF16, kind="ExternalInput")
    out_d = nc.dram_tensor("out", [B, T_DEC, VS], F32, kind="ExternalOutput")

    ag1i = [nc.dram_tensor(f"ag1i{t}", [128, 64], BF16) for t in range(T_DEC)]
    ag1o = [
        nc.dram_tensor(f"ag1o{t}", [NC * 128, 64], BF16, addr_space="Shared")
        for t in range(T_DEC)
    ]
    ag2i = [nc.dram_tensor(f"ag2i{t}", [128, 64], BF16) for t in range(T_DEC)]
    ag2o = [
        nc.dram_tensor(f"ag2o{t}", [NC * 128, 64], BF16, addr_space="Shared")
        for t in range(T_DEC)
    ]
    rg = [list(range(NC))]

    with tile.TileContext(nc) as tc:
        with (
            tc.tile_pool(name="one", bufs=1) as one,
            tc.tile_pool(name="work", bufs=2) as work,
            tc.tile_pool(name="gat", bufs=2) as gat,
            tc.tile_pool(name="hist", bufs=3) as hist,
            tc.tile_pool(name="ework", bufs=2) as ework,
            tc.tile_pool(name="zps", bufs=1, space="PSUM") as zps,
            tc.tile_pool(name="tps", bufs=2, space="PSUM") as tps,
            tc.tile_pool(name="aps", bufs=1, space="PSUM") as aps,
            tc.tile_pool(name="eps", bufs=2, space="PSUM") as eps,
            tc.tile_pool(name="fps", bufs=1, space="PSUM") as fps,
        ):
            # ---------------- resident tiles ----------------
            wz = one.tile([128, 16, 512], BF16)
            nc.sync.dma_start(wz[:], wz_d.rearrange("k p n -> p k n"))
            keysK = one.tile([128, KC, BL, T_IN], BF16)
            nc.sync.dma_start(keysK[:], keysk_d[:])
            memPK = one.tile([128, NPAIR, KC, 128], F16)
            nc.sync.dma_start(memPK[:], mempk_d[:])
            isel = one.tile([B, BL], F32, tag="isel")
            nc.gpsimd.dma_start(isel[:], isel_d[:])
            idf = one.tile([64, 64], F32, tag="idf")
            nc.gpsimd.dma_start(idf[:], idf_d[:])
            cst = one.tile([B, 128], F32, tag="cst")
            nc.gpsimd.dma_start(cst[:], c0_d[:])
            wa_sb = one.tile([128, 16, H], BF16)
            nc.sync.dma_start(wa_sb[:], wa_d.rearrange("k p h -> p k h"))
            fcw_sb = one.tile([128, KC, VS], BF16)
            nc.sync.dma_start(fcw_sb[:], fcw_d.rearrange("k p v -> p k v"))
            identb = one.tile([128, 128], BF16)
            nc.gpsimd.dma_start(identb[:], idb_d[:])
            fcbR = one.tile([128, VS], F32)
            nc.gpsimd.dma_start(fcbR[:], fcbr_d[:])

            h2T = one.tile([128, KC, 64], BF16, tag="h2T")
            nc.vector.memset(h2T[:], 0.0)
            ctxT = one.tile([128, KC, NC, BL], BF16, tag="ctxT")
            nc.vector.memset(ctxT[:], 0.0)
            alignZ = one.tile([128, BL], F16, tag="alignZ")
            nc.vector.memset(alignZ[:], 0.0)
            ones64 = one.tile([B, 1], F32, tag="ones64")
            nc.vector.memset(ones64[:], 1.0)
            onesr = one.tile([1, B], F32, tag="onesr")
            nc.vector.memset(onesr[:], 1.0)

            # ------------- epilogue chunk machinery -------------
            epiq = deque()

            def pump(n):
                k = 0
                while epiq and k < n:
                    epiq.popleft()()
                    k += 1

            def make_pair_chunks(t0, nsteps, h2p, ctxp):
                M = 64 * nsteps
                a2sb = ework.tile([128, H], BF16, tag="a2sb",
                                  name=f"a2sb{t0}")
                a2t = ework.tile([128, KC, 128], BF16, tag="a2t",
                                 name=f"a2t{t0}")

                def a2_chunk(n):
                    def go():
                        a2c = eps.tile([128, 512], F32, tag="a2c",
                                       name=f"a2c{t0}_{n}")
                        for k in range(16):
                            if k < KC:
                                lhs = h2p[:, k, 0:nsteps, :].rearrange(
                                    "p s b -> p (s b)")
                            else:
                                lhs = ctxp[:, k - KC, 0:nsteps, :].rearrange(
                                    "p s b -> p (s b)")
                            nc.tensor.matmul(
                                a2c[0:M, :], lhs,
                                wa_sb[:, k, n * 512:(n + 1) * 512],
                                start=(k == 0), stop=(k == 15),
                            )
                        nc.scalar.copy(a2sb[0:M, n * 512:(n + 1) * 512],
                                       a2c[0:M, :])
                    return go

                def tr_chunk():
                    for c2 in range(KC):
                        tpe = tps.tile([128, 128], BF16, tag="tp",
                                       name=f"a2tp{t0}_{c2}")
                        nc.tensor.transpose(
                            tpe[:, 0:M], a2sb[0:M, c2 * 128:(c2 + 1) * 128],
                            identb[0:M, 0:M])
                        nc.scalar.copy(a2t[:, c2, 0:M], tpe[:, 0:M])

                def fc_chunk(g):
                    def go():
                        lg = fps.tile([128, GS], F32, tag="lg",
                                      name=f"lg{t0}_{g}")
                        for k in range(KC):
                            nc.tensor.matmul(
                                lg[0:M, :], a2t[:, k, 0:M],
                                fcw_sb[:, k, g * GS:(g + 1) * GS],
                                start=(k == 0), stop=(k == KC - 1),
                            )
                        lgs = ework.tile([128, GS], F32, tag="lgs",
                                         name=f"lgs{t0}_{g}")
                        nc.vector.scalar_tensor_tensor(
                            lgs[0:M, :], lg[0:M, :], 1.0,
                            fcbR[0:M, g * GS:(g + 1) * GS], MULT, ADD)
                        for i in range(nsteps):
                            nc.gpsimd.dma_start(
                                out_d[:, t0 + i, g * GS:(g + 1) * GS],
                                lgs[i * 64:(i + 1) * 64, :],
                            )
                    return go

                epiq.append(a2_chunk(0))
                epiq.append(a2_chunk(1))
                epiq.append(tr_chunk)
                for g in range(8):
                    epiq.append(fc_chunk(g))

            # ================= decode loop =================
            h2p_cur = ctxp_cur = None
            for t in range(T_DEC):
                xpt = work.tile([B, 512], BF16, tag="xp")
                nc.sync.dma_start(xpt[:], xp_d[t])

                # z = [H*,ctx] @ wz  (k 0..7: H* tiles; 8..15: ctx tiles)
                zq = zps.tile([B, 512], F32, tag="zq")
                for k in range(16):
                    if k < KC:
                        lhs = h2T[:, k]
                    else:
                        lhs = ctxT[:, k - KC].rearrange("p q j -> p (q j)")
                    nc.tensor.matmul(
                        zq[:], lhs, wz[:, k],
                        start=(k == 0), stop=(k == 15),
                    )
                pump(2)

                # gates: layout [i|f|o|g] x128; H* = 2h, D = 2c
                z2 = gat.tile([B, 512], F32, tag="z2")
                nc.vector.scalar_tensor_tensor(z2[:], zq[:], 1.0, xpt[:], MULT, ADD)
                tio = gat.tile([B, 384], F32, tag="tio")
                nc.scalar.activation(tio[:], z2[:, 0:384], TANH, scale=0.5)
                tg = gat.tile([B, 128], F32, tag="tg")
                nc.scalar.activation(tg[:], z2[:, 384:512], TANH)
                av = gat.tile([B, 128], F32, tag="av")
                nc.vector.scalar_tensor_tensor(av[:], tio[:, 128:256], 1.0, cst[:], ADD, MULT)
                bv = gat.tile([B, 128], F32, tag="bv")
                nc.vector.scalar_tensor_tensor(bv[:], tio[:, 0:128], 1.0, tg[:], ADD, MULT)
                nc.vector.scalar_tensor_tensor(cst[:], av[:], 0.5, bv[:], MULT, ADD)
                tc2 = gat.tile([B, 128], F32, tag="tc2")
                nc.scalar.activation(tc2[:], cst[:], TANH, scale=0.5)
                h2s = gat.tile([B, 128], F32, tag="h2s")
                nc.vector.scalar_tensor_tensor(h2s[:], tio[:, 256:384], 1.0, tc2[:], ADD, MULT)

                # transpose own H* slice -> [128, 64] bf16, stage + AG1
                tp = tps.tile([128, 128], F32, tag="tp", name=f"htp{t}")
                nc.tensor.transpose(tp[:, 0:64], h2s[:], idf[:])
                h2sT = work.tile([128, 64], BF16, tag="h2sT")
                nc.scalar.copy(h2sT[:], tp[:, 0:64])
                nc.sync.dma_start(ag1i[t][:], h2sT[:])
                nc.gpsimd.collective_compute(
                    "AllGather", mybir.AluOpType.bypass,
                    replica_groups=rg,
                    ins=[ag1i[t][:]], outs=[ag1o[t][:]],
                )
                nc.scalar.dma_start(
                    h2T[:], ag1o[t].rearrange("(q p) b -> p q b", q=NC)
                )
                pump(4)

                # ---- scores for own 8 samples via cross + diag-select ----
                crossP = aps.tile([64, 512], F32, tag="crossP")
                for c in range(KC):
                    nc.tensor.matmul(
                        crossP[:], h2T[:, c],
                        keysK[:, c].rearrange("p j t -> p (j t)"),
                        start=(c == 0), stop=(c == KC - 1),
                    )
                ecr = gat.tile([64, 512], F32, tag="ecr")
                nc.scalar.activation(ecr[:], crossP[:], EXP)
                combo = aps.tile([128, KC * BL + 3 * BL], F32, tag="combo")
                ctxPS = combo[:, 0:KC * BL].rearrange("p (c j) -> p c j", c=KC)
                smx = combo[0:64, KC * BL:]
                for j in range(BL):
                    nc.tensor.matmul(
                        smx[:, j:j + 1], ecr[:, j * 64:(j + 1) * 64],
                        isel[:, j:j + 1], start=True, stop=True,
                    )
                e8 = gat.tile([64, BL], F32, tag="e8")
                nc.vector.tensor_copy(e8[:], smx[:, 0:BL])
                nc.tensor.matmul(smx[0:1, BL:2 * BL], ones64[:], e8[:],
                                 start=True, stop=True)
                r18 = gat.tile([1, BL], F32, tag="r18")
                nc.vector.reciprocal(r18[:], smx[0:1, BL:2 * BL])
                nc.tensor.matmul(smx[:, 2 * BL:3 * BL], onesr[:], r18[:],
                                 start=True, stop=True)
                a8 = gat.tile([64, BL], F16, tag="a8")
                nc.vector.tensor_mul(a8[:], e8[:], smx[:, 2 * BL:3 * BL])
                # scatter: even own-samples -> upper half, odd -> lower half
                nc.vector.tensor_copy(
                    alignZ[0:64, :].rearrange("p (pr two) -> p pr two", two=2)[:, :, 0],
                    a8[:].rearrange("p (pr two) -> p pr two", two=2)[:, :, 0],
                )
                nc.vector.tensor_copy(
                    alignZ[64:128, :].rearrange("p (pr two) -> p pr two", two=2)[:, :, 1],
                    a8[:].rearrange("p (pr two) -> p pr two", two=2)[:, :, 1],
                )

                # ---- ctx (own batches, pair-packed block-diag) ----
                for pr in range(NPAIR):
                    for c in range(KC):
                        nc.tensor.matmul(
                            ctxPS[:, c, 2 * pr:2 * pr + 2],
                            memPK[:, pr, c, :],
                            alignZ[:, 2 * pr:2 * pr + 2],
                            start=True, stop=True,
                        )
                ctxo = work.tile([128, KC, BL], BF16, tag="ctxo")
                nc.scalar.copy(ctxo[:], ctxPS[:])
                nc.sync.dma_start(ag2i[t][:], ctxo[:].rearrange("p c j -> p (c j)"))
                nc.gpsimd.collective_compute(
                    "AllGather", mybir.AluOpType.bypass,
                    replica_groups=rg,
                    ins=[ag2i[t][:]], outs=[ag2o[t][:]],
                )
                nc.scalar.dma_start(
                    ctxT[:],
                    ag2o[t].rearrange("(q p) (c j) -> p c q j", q=NC, c=KC),
                )
                pump(5)

                # ---- pair history (SBUF) + epilogue enqueue ----
                if t % 2 == 0:
                    h2p_cur = hist.tile([128, KC, 2, 64], BF16, tag="h2p",
                                        name=f"h2p{t}")
                    ctxp_cur = hist.tile([128, KC, 2, 64], BF16, tag="ctxp",
                                         name=f"ctxp{t}")
                slot = t % 2
                nc.vector.tensor_copy(h2p_cur[:, :, slot, :], h2T[:])
                nc.vector.tensor_copy(
                    ctxp_cur[:, :, slot, :],
                    ctxT[:].rearrange("p c q j -> p c (q j)"))
                if t % 2 == 1:
                    make_pair_chunks(t - 1, 2, h2p_cur, ctxp_cur)
                elif t == T_DEC - 1:
                    make_pair_chunks(t, 1, h2p_cur, ctxp_cur)

            # drain remaining epilogue work
            pump(len(epiq))

    nc.finalize()
    return nc


def _prep_inputs(inputs):
    bfnp = mybir.dt.np(BF16)
    f32 = lambda x: np.asarray(x, dtype=np.float32)
    tokens = np.asarray(inputs["tokens"])
    memory = f32(inputs["memory"])
    enc_h = f32(inputs["enc_h"])
    enc_c = f32(inputs["enc_c"])
    emb = f32(inputs["emb"])
    Wm = f32(inputs["Wm"])
    Wa = f32(inputs["Wa"])
    lstm_k = f32(inputs["lstm_k"])
    lstm_r = f32(inputs["lstm_r"])
    lstm_b = f32(inputs["lstm_b"])
    fc_w = f32(inputs["fc_w"])
    fc_b = f32(inputs["fc_b"])

    Wk_x = lstm_k[:E]
    Wk_a = lstm_k[E:]
    Rp = Wa[:H] @ Wk_a + lstm_r
    Cp = Wa[H:] @ Wk_a
    wz_full = np.concatenate([0.5 * Rp, Cp], axis=0)       # [2048, 4096]
    xs = emb[tokens]                                        # [B, T, E]
    xpb = (xs @ Wk_x + lstm_b).transpose(1, 0, 2).copy()    # [T, B, 4096]
    xpb[0] += enc_h @ lstm_r                                # t=0 folding fix
    keys = memory @ (0.5 * Wm)                              # [B, T_in, H]
    wa_eff = np.concatenate([0.5 * Wa[:H], Wa[H:]], axis=0)

    common = dict(
        idf=np.eye(64, dtype=np.float32),
        idb=np.eye(128, dtype=np.float32).astype(bfnp),
        wa=np.ascontiguousarray(wa_eff.reshape(16, 128, H)).astype(bfnp),
    )
    maps = []
    for r in range(NC):
        cols = np.concatenate([
            np.arange(r * 128, r * 128 + 128),
            H + np.arange(r * 128, r * 128 + 128),
            3 * H + np.arange(r * 128, r * 128 + 128),
            2 * H + np.arange(r * 128, r * 128 + 128),
        ])
        wz_r = np.ascontiguousarray(wz_full[:, cols]).reshape(16, 128, 512)
        xp_r = np.ascontiguousarray(xpb[:, :, cols])
        own = slice(r * BL, (r + 1) * BL)
        kk = keys[own].transpose(2, 0, 1)                   # [H, 8, T_in]
        keysK_r = np.ascontiguousarray(
            kk.reshape(KC, 128, BL, T_IN).transpose(1, 0, 2, 3))
        m = memory[own]                                     # [8, T_in, H]
        memPK_r = np.zeros((128, NPAIR, KC, 128), np.float16)
        for j in range(BL):
            memPK_r[(j % 2) * 64:(j % 2) * 64 + 64, j // 2] = (
                m[j].reshape(T_IN, KC, 128))
        isel_r = np.zeros((B, BL), np.float32)
        isel_r[r * BL + np.arange(BL), np.arange(BL)] = 1.0
        maps.append(dict(
            common,
            wz=wz_r.astype(bfnp),
            xp=xp_r.astype(bfnp),
            c0=np.ascontiguousarray(2.0 * enc_c[:, r * 128:(r + 1) * 128]),
            keysk=keysK_r.astype(bfnp),
            mempk=memPK_r,
            isel=isel_r,
            fcw=np.ascontiguousarray(
                fc_w[:, r * VS:(r + 1) * VS]).reshape(KC, 128, VS).astype(bfnp),
            fcbr=np.ascontiguousarray(
                np.broadcast_to(fc_b[r * VS:(r + 1) * VS], (128, VS)), np.float32),
        ))
    return maps


def kernel(**inputs):
    if "nc" not in _CACHE:
        _CACHE["nc"] = _build()
    nc = _CACHE["nc"]
    maps = _prep_inputs(inputs)
    res = run_bass_kernel_spmd(nc, maps, list(range(NC)))
    global LAST_RESULT
    LAST_RESULT = res
    out = np.concatenate([res.results[r]["out"] for r in range(NC)], axis=2)
    return out


LAST_RESULT = None


# revision 12
# speedup vs baseline: 1.4373x; 1.0638x over previous
"""Trainium2 Bass kernel for the Luong-attention LSTM decoder (nn_Decoder).

8-core strategy (v2, sharded recurrence + interleaved epilogue):
- Host folds Wa into the recurrence: z = [H*, ctx] @ wz + xp with
  wz = [0.5*(Wa_top@Wk_a + lstm_r); Wa_bot@Wk_a], xp precomputed (+bias,
  t=0 correction). State H* = 2*h2 (tanh-only gates, no ACT table
  switches); consumers pre-scaled by 0.5 on host.
- z matmul + gates column-sharded 8-way: core r computes gate columns
  [i_r|f_r|o_r|g_r] (128 each), keeps its c-state slice. AllGather #1
  reassembles transposed H*; AllGather #2 reassembles ctx. Single-DMA
  gathers.
- Attention batch-sharded (8 samples/core): cross-scores via 8 matmuls +
  matmul diagonal-select (SPMD-uniform via per-core one-hot isel).
- Epilogue (attn2 = [H*,ctx]@wa_eff, logits vocab-sharded 4000/core) is
  chunked and pumped into the AG-wait windows of later steps, keeping the
  PE HAM-warm and removing the serial tail. Pair history kept in SBUF.
"""
import sys

sys.path.insert(0, "/opt/trn_rl_repo")

from collections import deque

import numpy as np
import concourse.bass as bass
import concourse.tile as tile
from concourse import bacc, mybir
from concourse.bass_utils import run_bass_kernel_spmd

B, T_IN, T_DEC = 64, 64, 47
V, E, H = 32000, 512, 1024
NC = 8
BL = B // NC
VS = V // NC
KC = H // 128
NPAIR = BL // 2
GS = VS // 8
NPAIRS_T = (T_DEC + 1) // 2
F32 = mybir.dt.float32
F16 = mybir.dt.float16
BF16 = mybir.dt.bfloat16
TANH = mybir.ActivationFunctionType.Tanh
EXP = mybir.ActivationFunctionType.Exp
MULT = mybir.AluOpType.mult
ADD = mybir.AluOpType.add

_CACHE = {}


def _build():
    nc = bacc.Bacc(None, target_bir_lowering=False)

    wz_d = nc.dram_tensor("wz", [16, 128, 512], BF16, kind="ExternalInput")
    xp_d = nc.dram_tensor("xp", [T_DEC, B, 512], BF16, kind="ExternalInput")
    c0_d = nc.dram_tensor("c0", [B, 128], F32, kind="ExternalInput")
    keysk_d = nc.dram_tensor("keysk", [128, KC, BL, T_IN], BF16, kind="ExternalInput")
    mempk_d = nc.dram_tensor("mempk", [128, NPAIR, KC, 128], F16, kind="ExternalInput")
    isel_d = nc.dram_tensor("isel", [B, BL], F32, kind="ExternalInput")
    idf_d = nc.dram_tensor("idf", [64, 64], F32, kind="ExternalInput")
    wa_d = nc.dram_tensor("wa", [16, 128, H], BF16, kind="ExternalInput")
    fcw_d = nc.dram_tensor("fcw", [KC, 128, VS], BF16, kind="ExternalInput")
    fcbr_d = nc.dram_tensor("fcbr", [128, VS], F32, kind="ExternalInput")
    idb_d = nc.dram_tensor("idb", [128, 128], BF16, kind="ExternalInput")
    out_d = nc.dram_tensor("out", [B, T_DEC, VS], F32, kind="ExternalOutput")

    ag1i = [nc.dram_tensor(f"ag1i{t}", [64, 128], BF16) for t in range(T_DEC)]
    ag1o = [
        nc.dram_tensor(f"ag1o{t}", [NC * 64, 128], BF16, addr_space="Shared")
        for t in range(T_DEC)
    ]
    ag2i = [nc.dram_tensor(f"ag2i{t}", [128, 64], BF16) for t in range(T_DEC)]
    ag2o = [
        nc.dram_tensor(f"ag2o{t}", [NC * 128, 64], BF16, addr_space="Shared")
        for t in range(T_DEC)
    ]
    rg = [list(range(NC))]

    with tile.TileContext(nc) as tc:
        with (
            tc.tile_pool(name="one", bufs=1) as one,
            tc.tile_pool(name="work", bufs=2) as work,
            tc.tile_pool(name="gat", bufs=2) as gat,
            tc.tile_pool(name="hist", bufs=3) as hist,
            tc.tile_pool(name="ework", bufs=2) as ework,
            tc.tile_pool(name="zps", bufs=1, space="PSUM") as zps,
            tc.tile_pool(name="tps", bufs=2, space="PSUM") as tps,
            tc.tile_pool(name="aps", bufs=1, space="PSUM") as aps,
            tc.tile_pool(name="eps", bufs=2, space="PSUM") as eps,
            tc.tile_pool(name="fps", bufs=1, space="PSUM") as fps,
        ):
            # ---------------- resident tiles ----------------
            wz = one.tile([128, 16, 512], BF16)
            nc.sync.dma_start(wz[:], wz_d.rearrange("k p n -> p k n"))
            keysK = one.tile([128, KC, BL, T_IN], BF16)
            nc.sync.dma_start(keysK[:], keysk_d[:])
            memPK = one.tile([128, NPAIR, KC, 128], F16)
            nc.sync.dma_start(memPK[:], mempk_d[:])
            isel = one.tile([B, BL], F32, tag="isel")
            nc.sync.dma_start(isel[:], isel_d[:])
            idf = one.tile([64, 64], F32, tag="idf")
            nc.sync.dma_start(idf[:], idf_d[:])
            cst = one.tile([B, 128], F32, tag="cst")
            nc.sync.dma_start(cst[:], c0_d[:])
            wa_sb = one.tile([128, 16, H], BF16)
            nc.sync.dma_start(wa_sb[:], wa_d.rearrange("k p h -> p k h"))
            fcw_sb = one.tile([128, KC, VS], BF16)
            nc.sync.dma_start(fcw_sb[:], fcw_d.rearrange("k p v -> p k v"))
            identb = one.tile([128, 128], BF16)
            nc.sync.dma_start(identb[:], idb_d[:])
            fcbR = one.tile([128, VS], F32)
            nc.sync.dma_start(fcbR[:], fcbr_d[:])

            h2T = one.tile([128, KC, 64], BF16, tag="h2T")
            nc.vector.memset(h2T[:], 0.0)
            ctxT = one.tile([128, NC, KC, BL], BF16, tag="ctxT")
            nc.vector.memset(ctxT[:], 0.0)
            ctxTz = one.tile([128, KC, NC, BL], BF16, tag="ctxTz")
            nc.vector.memset(ctxTz[:], 0.0)
            alignZ = one.tile([128, BL], F16, tag="alignZ")
            nc.vector.memset(alignZ[:], 0.0)
            ones64 = one.tile([B, 1], F32, tag="ones64")
            nc.vector.memset(ones64[:], 1.0)
            onesr = one.tile([1, B], F32, tag="onesr")
            nc.vector.memset(onesr[:], 1.0)

            # ------------- epilogue chunk machinery -------------
            epiq = deque()

            def pump(n):
                k = 0
                while epiq and k < n:
                    epiq.popleft()()
                    k += 1

            def make_pair_chunks(t0, nsteps, h2p, ctxp):
                M = 64 * nsteps
                a2sb = ework.tile([128, H], BF16, tag="a2sb",
                                  name=f"a2sb{t0}")
                a2t = ework.tile([128, KC, 128], BF16, tag="a2t",
                                 name=f"a2t{t0}")

                def a2_chunk(n):
                    def go():
                        a2c = eps.tile([128, 512], F32, tag="a2c",
                                       name=f"a2c{t0}_{n}")
                        for k in range(16):
                            if k < KC:
                                lhs = h2p[:, k, 0:nsteps, :].rearrange(
                                    "p s b -> p (s b)")
                            else:
                                lhs = ctxp[:, k - KC, 0:nsteps, :].rearrange(
                                    "p s b -> p (s b)")
                            nc.tensor.matmul(
                                a2c[0:M, :], lhs,
                                wa_sb[:, k, n * 512:(n + 1) * 512],
                                start=(k == 0), stop=(k == 15),
                            )
                        nc.scalar.copy(a2sb[0:M, n * 512:(n + 1) * 512],
                                       a2c[0:M, :])
                    return go

                def tr_chunk():
                    for c2 in range(KC):
                        tpe = tps.tile([128, 128], BF16, tag="tp",
                                       name=f"a2tp{t0}_{c2}")
                        nc.tensor.transpose(
                            tpe[:, 0:M], a2sb[0:M, c2 * 128:(c2 + 1) * 128],
                            identb[0:M, 0:M])
                        nc.scalar.copy(a2t[:, c2, 0:M], tpe[:, 0:M])

                def fc_chunk(g):
                    def go():
                        lg = fps.tile([128, GS], F32, tag="lg",
                                      name=f"lg{t0}_{g}")
                        for k in range(KC):
                            nc.tensor.matmul(
                                lg[0:M, :], a2t[:, k, 0:M],
                                fcw_sb[:, k, g * GS:(g + 1) * GS],
                                start=(k == 0), stop=(k == KC - 1),
                            )
                        lgs = ework.tile([128, GS], F32, tag="lgs",
                                         name=f"lgs{t0}_{g}")
                        nc.vector.scalar_tensor_tensor(
                            lgs[0:M, :], lg[0:M, :], 1.0,
                            fcbR[0:M, g * GS:(g + 1) * GS], MULT, ADD)
                        for i in range(nsteps):
                            nc.sync.dma_start(
                                out_d[:, t0 + i, g * GS:(g + 1) * GS],
                                lgs[i * 64:(i + 1) * 64, :],
                            )
                    return go

                epiq.append(a2_chunk(0))
                epiq.append(a2_chunk(1))
                epiq.append(tr_chunk)
                for g in range(8):
                    epiq.append(fc_chunk(g))

            # ================= decode loop =================
            h2p_cur = ctxp_cur = None
            for t in range(T_DEC):
                xpt = work.tile([B, 512], BF16, tag="xp")
                nc.sync.dma_start(xpt[:], xp_d[t])

                # z = [H*,ctx] @ wz  (k 0..7: H* tiles; 8..15: ctx tiles)
                zq = zps.tile([B, 512], F32, tag="zq")
                for k in range(16):
                    if k < KC:
                        lhs = h2T[:, k]
                    else:
                        lhs = ctxTz[:, k - KC].rearrange("p q j -> p (q j)")
                    nc.tensor.matmul(
                        zq[:], lhs, wz[:, k],
                        start=(k == 0), stop=(k == 15),
                    )
                pump(2)

                # gates: layout [i|f|o|g] x128; H* = 2h, D = 2c
                z2 = gat.tile([B, 512], F32, tag="z2")
                nc.vector.scalar_tensor_tensor(z2[:], zq[:], 1.0, xpt[:], MULT, ADD)
                tio = gat.tile([B, 384], F32, tag="tio")
                nc.scalar.activation(tio[:], z2[:, 0:384], TANH, scale=0.5)
                tg = gat.tile([B, 128], F32, tag="tg")
                nc.scalar.activation(tg[:], z2[:, 384:512], TANH)
                av = gat.tile([B, 128], F32, tag="av")
                nc.vector.scalar_tensor_tensor(av[:], tio[:, 128:256], 1.0, cst[:], ADD, MULT)
                bv = gat.tile([B, 128], F32, tag="bv")
                nc.vector.scalar_tensor_tensor(bv[:], tio[:, 0:128], 1.0, tg[:], ADD, MULT)
                nc.vector.scalar_tensor_tensor(cst[:], av[:], 0.5, bv[:], MULT, ADD)
                tc2 = gat.tile([B, 128], F32, tag="tc2")
                nc.scalar.activation(tc2[:], cst[:], TANH, scale=0.5)
                h2s = gat.tile([B, 128], BF16, tag="h2s")
                nc.vector.scalar_tensor_tensor(h2s[:], tio[:, 256:384], 1.0, tc2[:], ADD, MULT)

                # stage own H* slice [64, 128]; transpose happens in the
                # gather DMA (xbar)
                nc.sync.dma_start(ag1i[t][:], h2s[:])
                nc.gpsimd.collective_compute(
                    "AllGather", mybir.AluOpType.bypass,
                    replica_groups=rg,
                    ins=[ag1i[t][:]], outs=[ag1o[t][:]],
                )
                pump(2)
                nc.scalar.dma_start(
                    h2T[:].rearrange("p q b -> p (q b)"), ag1o[t][:],
                    transpose=True,
                )

                # ---- scores for own 8 samples via cross + diag-select ----
                crossP = aps.tile([64, 512], F32, tag="crossP")
                for c in range(KC):
                    nc.tensor.matmul(
                        crossP[:], h2T[:, c],
                        keysK[:, c].rearrange("p j t -> p (j t)"),
                        start=(c == 0), stop=(c == KC - 1),
                    )
                ecr = gat.tile([64, 512], F32, tag="ecr")
                nc.scalar.activation(ecr[:], crossP[:], EXP)
                combo = aps.tile([128, KC * BL + 3 * BL], F32, tag="combo")
                ctxPS = combo[:, 0:KC * BL].rearrange("p (c j) -> p c j", c=KC)
                smx = combo[0:64, KC * BL:]
                for j in range(BL):
                    nc.tensor.matmul(
                        smx[:, j:j + 1], ecr[:, j * 64:(j + 1) * 64],
                        isel[:, j:j + 1], start=True, stop=True,
                    )
                e8 = gat.tile([64, BL], F32, tag="e8")
                nc.vector.tensor_copy(e8[:], smx[:, 0:BL])
                nc.tensor.matmul(smx[0:1, BL:2 * BL], ones64[:], e8[:],
                                 start=True, stop=True)
                r18 = gat.tile([1, BL], F32, tag="r18")
                nc.vector.reciprocal(r18[:], smx[0:1, BL:2 * BL])
                nc.tensor.matmul(smx[:, 2 * BL:3 * BL], onesr[:], r18[:],
                                 start=True, stop=True)
                a8 = gat.tile([64, BL], F16, tag="a8")
                nc.vector.tensor_mul(a8[:], e8[:], smx[:, 2 * BL:3 * BL])
                # scatter: even own-samples -> upper half, odd -> lower half
                nc.vector.tensor_copy(
                    alignZ[0:64, :].rearrange("p (pr two) -> p pr two", two=2)[:, :, 0],
                    a8[:].rearrange("p (pr two) -> p pr two", two=2)[:, :, 0],
                )
                nc.vector.tensor_copy(
                    alignZ[64:128, :].rearrange("p (pr two) -> p pr two", two=2)[:, :, 1],
                    a8[:].rearrange("p (pr two) -> p pr two", two=2)[:, :, 1],
                )

                # ---- ctx (own batches, pair-packed block-diag) ----
                for pr in range(NPAIR):
                    for c in range(KC):
                        nc.tensor.matmul(
                            ctxPS[:, c, 2 * pr:2 * pr + 2],
                            memPK[:, pr, c, :],
                            alignZ[:, 2 * pr:2 * pr + 2],
                            start=True, stop=True,
                        )
                ctxo = work.tile([128, KC, BL], BF16, tag="ctxo")
                nc.scalar.copy(ctxo[:], ctxPS[:])
                nc.sync.dma_start(ag2i[t][:], ctxo[:].rearrange("p c j -> p (c j)"))
                nc.gpsimd.collective_compute(
                    "AllGather", mybir.AluOpType.bypass,
                    replica_groups=rg,
                    ins=[ag2i[t][:]], outs=[ag2o[t][:]],
                )
                pump(3)
                nc.scalar.dma_start(
                    ctxT[:],
                    ag2o[t].rearrange("(q p) (c j) -> p q c j", q=NC, c=KC),
                )
                nc.vector.tensor_copy(
                    ctxTz[:], ctxT[:].rearrange("p q c j -> p c q j"))

                # ---- pair history (SBUF) + epilogue enqueue ----
                if t % 2 == 0:
                    h2p_cur = hist.tile([128, KC, 2, 64], BF16, tag="h2p",
                                        name=f"h2p{t}")
                    ctxp_cur = hist.tile([128, KC, 2, 64], BF16, tag="ctxp",
                                         name=f"ctxp{t}")
                slot = t % 2
                nc.vector.tensor_copy(h2p_cur[:, :, slot, :], h2T[:])
                nc.vector.tensor_copy(
                    ctxp_cur[:, :, slot, :],
                    ctxTz[:].rearrange("p c q j -> p c (q j)"))
                if t % 2 == 1:
                    make_pair_chunks(t - 1, 2, h2p_cur, ctxp_cur)
                elif t == T_DEC - 1:
                    make_pair_chunks(t, 1, h2p_cur, ctxp_cur)

            # drain remaining epilogue work
            pump(len(epiq))

    nc.finalize()
    return nc


def _prep_inputs(inputs):
    bfnp = mybir.dt.np(BF16)
    f32 = lambda x: np.asarray(x, dtype=np.float32)
    tokens = np.asarray(inputs["tokens"])
    memory = f32(inputs["memory"])
    enc_h = f32(inputs["enc_h"])
    enc_c = f32(inputs["enc_c"])
    emb = f32(inputs["emb"])
    Wm = f32(inputs["Wm"])
    Wa = f32(inputs["Wa"])
    lstm_k = f32(inputs["lstm_k"])
    lstm_r = f32(inputs["lstm_r"])
    lstm_b = f32(inputs["lstm_b"])
    fc_w = f32(inputs["fc_w"])
    fc_b = f32(inputs["fc_b"])

    Wk_x = lstm_k[:E]
    Wk_a = lstm_k[E:]
    Rp = Wa[:H] @ Wk_a + lstm_r
    Cp = Wa[H:] @ Wk_a
    wz_full = np.concatenate([0.5 * Rp, Cp], axis=0)       # [2048, 4096]
    xs = emb[tokens]                                        # [B, T, E]
    xpb = (xs @ Wk_x + lstm_b).transpose(1, 0, 2).copy()    # [T, B, 4096]
    xpb[0] += enc_h @ lstm_r                                # t=0 folding fix
    keys = memory @ (0.5 * Wm)                              # [B, T_in, H]
    wa_eff = np.concatenate([0.5 * Wa[:H], Wa[H:]], axis=0)

    common = dict(
        idf=np.eye(64, dtype=np.float32),
        idb=np.eye(128, dtype=np.float32).astype(bfnp),
        wa=np.ascontiguousarray(wa_eff.reshape(16, 128, H)).astype(bfnp),
    )
    maps = []
    for r in range(NC):
        cols = np.concatenate([
            np.arange(r * 128, r * 128 + 128),
            H + np.arange(r * 128, r * 128 + 128),
            3 * H + np.arange(r * 128, r * 128 + 128),
            2 * H + np.arange(r * 128, r * 128 + 128),
        ])
        wz_r = np.ascontiguousarray(wz_full[:, cols]).reshape(16, 128, 512)
        xp_r = np.ascontiguousarray(xpb[:, :, cols])
        own = slice(r * BL, (r + 1) * BL)
        kk = keys[own].transpose(2, 0, 1)                   # [H, 8, T_in]
        keysK_r = np.ascontiguousarray(
            kk.reshape(KC, 128, BL, T_IN).transpose(1, 0, 2, 3))
        m = memory[own]                                     # [8, T_in, H]
        memPK_r = np.zeros((128, NPAIR, KC, 128), np.float16)
        for j in range(BL):
            memPK_r[(j % 2) * 64:(j % 2) * 64 + 64, j // 2] = (
                m[j].reshape(T_IN, KC, 128))
        isel_r = np.zeros((B, BL), np.float32)
        isel_r[r * BL + np.arange(BL), np.arange(BL)] = 1.0
        maps.append(dict(
            common,
            wz=wz_r.astype(bfnp),
            xp=xp_r.astype(bfnp),
            c0=np.ascontiguousarray(2.0 * enc_c[:, r * 128:(r + 1) * 128]),
            keysk=keysK_r.astype(bfnp),
            mempk=memPK_r,
            isel=isel_r,
            fcw=np.ascontiguousarray(
                fc_w[:, r * VS:(r + 1) * VS]).reshape(KC, 128, VS).astype(bfnp),
            fcbr=np.ascontiguousarray(
                np.broadcast_to(fc_b[r * VS:(r + 1) * VS], (128, VS)), np.float32),
        ))
    return maps


def kernel(**inputs):
    if "nc" not in _CACHE:
        _CACHE["nc"] = _build()
    nc = _CACHE["nc"]
    maps = _prep_inputs(inputs)
    res = run_bass_kernel_spmd(nc, maps, list(range(NC)))
    global LAST_RESULT
    LAST_RESULT = res
    out = np.concatenate([res.results[r]["out"] for r in range(NC)], axis=2)
    return out


LAST_RESULT = None


# revision 14
# speedup vs baseline: 1.5479x; 1.0770x over previous
"""Trainium2 Bass kernel for the Luong-attention LSTM decoder (nn_Decoder).

8-core strategy (v2, sharded recurrence + interleaved epilogue):
- Host folds Wa into the recurrence: z = [H*, ctx] @ wz + xp with
  wz = [0.5*(Wa_top@Wk_a + lstm_r); Wa_bot@Wk_a], xp precomputed (+bias,
  t=0 correction). State H* = 2*h2 (tanh-only gates, no ACT table
  switches); consumers pre-scaled by 0.5 on host.
- z matmul + gates column-sharded 8-way: core r computes gate columns
  [i_r|f_r|o_r|g_r] (128 each), keeps its c-state slice. AllGather #1
  reassembles transposed H*; AllGather #2 reassembles ctx. Single-DMA
  gathers.
- Attention batch-sharded (8 samples/core): cross-scores via 8 matmuls +
  matmul diagonal-select (SPMD-uniform via per-core one-hot isel).
- Epilogue (attn2 = [H*,ctx]@wa_eff, logits vocab-sharded 4000/core) is
  chunked and pumped into the AG-wait windows of later steps, keeping the
  PE HAM-warm and removing the serial tail. Pair history kept in SBUF.
"""
import sys

sys.path.insert(0, "/opt/trn_rl_repo")

from collections import deque

import numpy as np
import concourse.bass as bass
import concourse.tile as tile
from concourse import bacc, mybir
from concourse.bass_utils import run_bass_kernel_spmd

B, T_IN, T_DEC = 64, 64, 47
V, E, H = 32000, 512, 1024
NC = 8
BL = B // NC
VS = V // NC
KC = H // 128
NPAIR = BL // 2
GS = VS // 8
NPAIRS_T = (T_DEC + 1) // 2
F32 = mybir.dt.float32
F16 = mybir.dt.float16
BF16 = mybir.dt.bfloat16
TANH = mybir.ActivationFunctionType.Tanh
EXP = mybir.ActivationFunctionType.Exp
MULT = mybir.AluOpType.mult
ADD = mybir.AluOpType.add

_CACHE = {}


def _build():
    nc = bacc.Bacc(None, target_bir_lowering=False)

    wz_d = nc.dram_tensor("wz", [16, 128, 512], BF16, kind="ExternalInput")
    xp_d = nc.dram_tensor("xp", [T_DEC, B, 512], BF16, kind="ExternalInput")
    c0_d = nc.dram_tensor("c0", [B, 128], F32, kind="ExternalInput")
    keysk_d = nc.dram_tensor("keysk", [128, KC, BL, T_IN], BF16, kind="ExternalInput")
    mempk_d = nc.dram_tensor("mempk", [128, NPAIR, KC, 128], F16, kind="ExternalInput")
    isel_d = nc.dram_tensor("isel", [B, BL], F32, kind="ExternalInput")
    idf_d = nc.dram_tensor("idf", [64, 64], F32, kind="ExternalInput")
    wa_d = nc.dram_tensor("wa", [16, 128, H], BF16, kind="ExternalInput")
    fcw_d = nc.dram_tensor("fcw", [KC, 128, VS], BF16, kind="ExternalInput")
    fcbr_d = nc.dram_tensor("fcbr", [128, VS], F32, kind="ExternalInput")
    idb_d = nc.dram_tensor("idb", [128, 128], BF16, kind="ExternalInput")
    out_d = nc.dram_tensor("out", [B, T_DEC, VS], F32, kind="ExternalOutput")

    ag1i = [nc.dram_tensor(f"ag1i{t}", [64, 128], BF16) for t in range(T_DEC)]
    ag1o = [
        nc.dram_tensor(f"ag1o{t}", [NC * 64, 128], BF16, addr_space="Shared")
        for t in range(T_DEC)
    ]
    ag2i = [nc.dram_tensor(f"ag2i{t}", [128, 64], BF16) for t in range(T_DEC)]
    ag2o = [
        nc.dram_tensor(f"ag2o{t}", [NC * 128, 64], BF16, addr_space="Shared")
        for t in range(T_DEC)
    ]
    rg = [list(range(NC))]

    with tile.TileContext(nc) as tc:
        with (
            tc.tile_pool(name="one", bufs=1) as one,
            tc.tile_pool(name="work", bufs=2) as work,
            tc.tile_pool(name="gat", bufs=2) as gat,
            tc.tile_pool(name="hist", bufs=3) as hist,
            tc.tile_pool(name="ework", bufs=2) as ework,
            tc.tile_pool(name="zps", bufs=1, space="PSUM") as zps,
            tc.tile_pool(name="tps", bufs=2, space="PSUM") as tps,
            tc.tile_pool(name="aps", bufs=1, space="PSUM") as aps,
            tc.tile_pool(name="eps", bufs=2, space="PSUM") as eps,
            tc.tile_pool(name="fps", bufs=1, space="PSUM") as fps,
        ):
            # ---------------- resident tiles ----------------
            wz = one.tile([128, 16, 512], BF16)
            nc.sync.dma_start(wz[:], wz_d.rearrange("k p n -> p k n"))
            keysK = one.tile([128, KC, BL, T_IN], BF16)
            nc.sync.dma_start(keysK[:], keysk_d[:])
            memPK = one.tile([128, NPAIR, KC, 128], F16)
            nc.sync.dma_start(memPK[:], mempk_d[:])
            isel = one.tile([B, BL], F32, tag="isel")
            nc.sync.dma_start(isel[:], isel_d[:])
            idf = one.tile([64, 64], F32, tag="idf")
            nc.sync.dma_start(idf[:], idf_d[:])
            cst = one.tile([B, 128], F32, tag="cst")
            nc.sync.dma_start(cst[:], c0_d[:])
            wa_sb = one.tile([128, 16, H], BF16)
            nc.sync.dma_start(wa_sb[:], wa_d.rearrange("k p h -> p k h"))
            fcw_sb = one.tile([128, KC, VS], BF16)
            nc.sync.dma_start(fcw_sb[:], fcw_d.rearrange("k p v -> p k v"))
            identb = one.tile([128, 128], BF16)
            nc.sync.dma_start(identb[:], idb_d[:])
            fcbR = one.tile([128, VS], F32)
            nc.sync.dma_start(fcbR[:], fcbr_d[:])

            h2T = one.tile([128, KC, 64], BF16, tag="h2T")
            nc.vector.memset(h2T[:], 0.0)
            ctxT = one.tile([128, NC, KC, BL], BF16, tag="ctxT")
            nc.vector.memset(ctxT[:], 0.0)
            ctxTz = one.tile([128, KC, NC, BL], BF16, tag="ctxTz")
            nc.vector.memset(ctxTz[:], 0.0)
            alignZ = one.tile([128, BL], F16, tag="alignZ")
            nc.vector.memset(alignZ[:], 0.0)
            ones64 = one.tile([B, 1], F32, tag="ones64")
            nc.vector.memset(ones64[:], 1.0)
            onesr = one.tile([1, B], F32, tag="onesr")
            nc.vector.memset(onesr[:], 1.0)

            # ------------- epilogue chunk machinery -------------
            epiq = deque()

            def pump(budget_us):
                spent = 0.0
                while epiq and spent < budget_us:
                    cost, go = epiq.popleft()
                    go()
                    spent += cost

            def make_pair_chunks(t0, nsteps, h2p, ctxp):
                M = 64 * nsteps
                a2sb = ework.tile([128, H], BF16, tag="a2sb",
                                  name=f"a2sb{t0}")
                a2t = ework.tile([128, KC, 128], BF16, tag="a2t",
                                 name=f"a2t{t0}")

                a2cs = {}

                def a2_chunk(n, half):
                    def go():
                        if n not in a2cs:
                            a2cs[n] = eps.tile([128, 512], F32, tag="a2c",
                                               name=f"a2c{t0}_{n}")
                        a2c = a2cs[n]
                        for k in range(half * 8, half * 8 + 8):
                            if k < KC:
                                lhs = h2p[:, k, 0:nsteps, :].rearrange(
                                    "p s b -> p (s b)")
                            else:
                                lhs = ctxp[:, k - KC, 0:nsteps, :].rearrange(
                                    "p s b -> p (s b)")
                            nc.tensor.matmul(
                                a2c[0:M, :], lhs,
                                wa_sb[:, k, n * 512:(n + 1) * 512],
                                start=(k == 0), stop=(k == 15),
                            )
                        if half == 1:
                            nc.scalar.copy(a2sb[0:M, n * 512:(n + 1) * 512],
                                           a2c[0:M, :])
                    return go

                def tr_chunk():
                    for c2 in range(KC):
                        tpe = tps.tile([128, 128], BF16, tag="tp",
                                       name=f"a2tp{t0}_{c2}")
                        nc.tensor.transpose(
                            tpe[:, 0:M], a2sb[0:M, c2 * 128:(c2 + 1) * 128],
                            identb[0:M, 0:M])
                        nc.scalar.copy(a2t[:, c2, 0:M], tpe[:, 0:M])

                def fc_chunk(g):
                    def go():
                        lg = fps.tile([128, GS], F32, tag="lg",
                                      name=f"lg{t0}_{g}")
                        for k in range(KC):
                            nc.tensor.matmul(
                                lg[0:M, :], a2t[:, k, 0:M],
                                fcw_sb[:, k, g * GS:(g + 1) * GS],
                                start=(k == 0), stop=(k == KC - 1),
                            )
                        lgs = ework.tile([128, GS], F32, tag="lgs",
                                         name=f"lgs{t0}_{g}")
                        nc.vector.scalar_tensor_tensor(
                            lgs[0:M, :], lg[0:M, :], 1.0,
                            fcbR[0:M, g * GS:(g + 1) * GS], MULT, ADD)
                        for i in range(nsteps):
                            nc.scalar.dma_start(
                                out_d[:, t0 + i, g * GS:(g + 1) * GS],
                                lgs[i * 64:(i + 1) * 64, :],
                            )
                    return go

                for n in range(2):
                    for half in range(2):
                        epiq.append((1.8, a2_chunk(n, half)))
                epiq.append((2.6, tr_chunk))
                for g in range(8):
                    epiq.append((1.8, fc_chunk(g)))

            # ================= decode loop =================
            h2p_cur = ctxp_cur = None
            for t in range(T_DEC):
                xpt = work.tile([B, 512], BF16, tag="xp")
                nc.sync.dma_start(xpt[:], xp_d[t])

                # z = [H*,ctx] @ wz  (k 0..7: H* tiles; 8..15: ctx tiles)
                zq = zps.tile([B, 512], F32, tag="zq")
                for k in range(16):
                    if k < KC:
                        lhs = h2T[:, k]
                    else:
                        lhs = ctxTz[:, k - KC].rearrange("p q j -> p (q j)")
                    nc.tensor.matmul(
                        zq[:], lhs, wz[:, k],
                        start=(k == 0), stop=(k == 15),
                    )
                pump(3.8)

                # gates: layout [i|f|o|g] x128; H* = 2h, D = 2c
                z2 = gat.tile([B, 512], F32, tag="z2")
                nc.vector.scalar_tensor_tensor(z2[:], zq[:], 1.0, xpt[:], MULT, ADD)
                tio = gat.tile([B, 384], F32, tag="tio")
                nc.scalar.activation(tio[:], z2[:, 0:384], TANH, scale=0.5)
                tg = gat.tile([B, 128], F32, tag="tg")
                nc.scalar.activation(tg[:], z2[:, 384:512], TANH)
                av = gat.tile([B, 128], F32, tag="av")
                nc.vector.scalar_tensor_tensor(av[:], tio[:, 128:256], 1.0, cst[:], ADD, MULT)
                bv = gat.tile([B, 128], F32, tag="bv")
                nc.vector.scalar_tensor_tensor(bv[:], tio[:, 0:128], 1.0, tg[:], ADD, MULT)
                nc.vector.scalar_tensor_tensor(cst[:], av[:], 0.5, bv[:], MULT, ADD)
                tc2 = gat.tile([B, 128], F32, tag="tc2")
                nc.scalar.activation(tc2[:], cst[:], TANH, scale=0.5)
                h2s = gat.tile([B, 128], BF16, tag="h2s")
                nc.vector.scalar_tensor_tensor(h2s[:], tio[:, 256:384], 1.0, tc2[:], ADD, MULT)

                # stage own H* slice [64, 128]; transpose happens in the
                # gather DMA (xbar)
                nc.sync.dma_start(ag1i[t][:], h2s[:])
                nc.gpsimd.collective_compute(
                    "AllGather", mybir.AluOpType.bypass,
                    replica_groups=rg,
                    ins=[ag1i[t][:]], outs=[ag1o[t][:]],
                )
                pump(5.5)
                nc.scalar.dma_start(
                    h2T[:].rearrange("p q b -> p (q b)"), ag1o[t][:],
                    transpose=True,
                )

                # ---- scores for own 8 samples via cross + diag-select ----
                crossP = aps.tile([64, 512], F32, tag="crossP")
                for c in range(KC):
                    nc.tensor.matmul(
                        crossP[:], h2T[:, c],
                        keysK[:, c].rearrange("p j t -> p (j t)"),
                        start=(c == 0), stop=(c == KC - 1),
                    )
                ecr = gat.tile([64, 512], F32, tag="ecr")
                nc.scalar.activation(ecr[:], crossP[:], EXP)
                combo = aps.tile([128, KC * BL + 3 * BL], F32, tag="combo")
                ctxPS = combo[:, 0:KC * BL].rearrange("p (c j) -> p c j", c=KC)
                smx = combo[0:64, KC * BL:]
                for j in range(BL):
                    nc.tensor.matmul(
                        smx[:, j:j + 1], ecr[:, j * 64:(j + 1) * 64],
                        isel[:, j:j + 1], start=True, stop=True,
                    )
                e8 = gat.tile([64, BL], F32, tag="e8")
                nc.vector.tensor_copy(e8[:], smx[:, 0:BL])
                nc.tensor.matmul(smx[0:1, BL:2 * BL], ones64[:], e8[:],
                                 start=True, stop=True)
                r18 = gat.tile([1, BL], F32, tag="r18")
                nc.vector.reciprocal(r18[:], smx[0:1, BL:2 * BL])
                nc.tensor.matmul(smx[:, 2 * BL:3 * BL], onesr[:], r18[:],
                                 start=True, stop=True)
                a8 = gat.tile([64, BL], F16, tag="a8")
                nc.vector.tensor_mul(a8[:], e8[:], smx[:, 2 * BL:3 * BL])
                # scatter: even own-samples -> upper half, odd -> lower half
                nc.vector.tensor_copy(
                    alignZ[0:64, :].rearrange("p (pr two) -> p pr two", two=2)[:, :, 0],
                    a8[:].rearrange("p (pr two) -> p pr two", two=2)[:, :, 0],
                )
                nc.vector.tensor_copy(
                    alignZ[64:128, :].rearrange("p (pr two) -> p pr two", two=2)[:, :, 1],
                    a8[:].rearrange("p (pr two) -> p pr two", two=2)[:, :, 1],
                )

                # ---- ctx (own batches, pair-packed block-diag) ----
                for pr in range(NPAIR):
                    for c in range(KC):
                        nc.tensor.matmul(
                            ctxPS[:, c, 2 * pr:2 * pr + 2],
                            memPK[:, pr, c, :],
                            alignZ[:, 2 * pr:2 * pr + 2],
                            start=True, stop=True,
                        )
                ctxo = work.tile([128, KC, BL], BF16, tag="ctxo")
                nc.scalar.copy(ctxo[:], ctxPS[:])
                nc.sync.dma_start(ag2i[t][:], ctxo[:].rearrange("p c j -> p (c j)"))
                nc.gpsimd.collective_compute(
                    "AllGather", mybir.AluOpType.bypass,
                    replica_groups=rg,
                    ins=[ag2i[t][:]], outs=[ag2o[t][:]],
                )
                pump(7.5)
                nc.scalar.dma_start(
                    ctxT[:],
                    ag2o[t].rearrange("(q p) (c j) -> p q c j", q=NC, c=KC),
                )
                nc.vector.tensor_copy(
                    ctxTz[:], ctxT[:].rearrange("p q c j -> p c q j"))

                # ---- pair history (SBUF) + epilogue enqueue ----
                if t % 2 == 0:
                    h2p_cur = hist.tile([128, KC, 2, 64], BF16, tag="h2p",
                                        name=f"h2p{t}")
                    ctxp_cur = hist.tile([128, KC, 2, 64], BF16, tag="ctxp",
                                         name=f"ctxp{t}")
                slot = t % 2
                nc.vector.tensor_copy(h2p_cur[:, :, slot, :], h2T[:])
                nc.vector.tensor_copy(
                    ctxp_cur[:, :, slot, :],
                    ctxTz[:].rearrange("p c q j -> p c (q j)"))
                if t % 2 == 1:
                    make_pair_chunks(t - 1, 2, h2p_cur, ctxp_cur)
                elif t == T_DEC - 1:
                    make_pair_chunks(t, 1, h2p_cur, ctxp_cur)

            # drain remaining epilogue work
            pump(1e9)

    nc.finalize()
    return nc


def _prep_inputs(inputs):
    bfnp = mybir.dt.np(BF16)
    f32 = lambda x: np.asarray(x, dtype=np.float32)
    tokens = np.asarray(inputs["tokens"])
    memory = f32(inputs["memory"])
    enc_h = f32(inputs["enc_h"])
    enc_c = f32(inputs["enc_c"])
    emb = f32(inputs["emb"])
    Wm = f32(inputs["Wm"])
    Wa = f32(inputs["Wa"])
    lstm_k = f32(inputs["lstm_k"])
    lstm_r = f32(inputs["lstm_r"])
    lstm_b = f32(inputs["lstm_b"])
    fc_w = f32(inputs["fc_w"])
    fc_b = f32(inputs["fc_b"])

    Wk_x = lstm_k[:E]
    Wk_a = lstm_k[E:]
    Rp = Wa[:H] @ Wk_a + lstm_r
    Cp = Wa[H:] @ Wk_a
    wz_full = np.concatenate([0.5 * Rp, Cp], axis=0)       # [2048, 4096]
    xs = emb[tokens]                                        # [B, T, E]
    xpb = (xs @ Wk_x + lstm_b).transpose(1, 0, 2).copy()    # [T, B, 4096]
    xpb[0] += enc_h @ lstm_r                                # t=0 folding fix
    keys = memory @ (0.5 * Wm)                              # [B, T_in, H]
    wa_eff = np.concatenate([0.5 * Wa[:H], Wa[H:]], axis=0)

    common = dict(
        idf=np.eye(64, dtype=np.float32),
        idb=np.eye(128, dtype=np.float32).astype(bfnp),
        wa=np.ascontiguousarray(wa_eff.reshape(16, 128, H)).astype(bfnp),
    )
    maps = []
    for r in range(NC):
        cols = np.concatenate([
            np.arange(r * 128, r * 128 + 128),
            H + np.arange(r * 128, r * 128 + 128),
            3 * H + np.arange(r * 128, r * 128 + 128),
            2 * H + np.arange(r * 128, r * 128 + 128),
        ])
        wz_r = np.ascontiguousarray(wz_full[:, cols]).reshape(16, 128, 512)
        xp_r = np.ascontiguousarray(xpb[:, :, cols])
        own = slice(r * BL, (r + 1) * BL)
        kk = keys[own].transpose(2, 0, 1)                   # [H, 8, T_in]
        keysK_r = np.ascontiguousarray(
            kk.reshape(KC, 128, BL, T_IN).transpose(1, 0, 2, 3))
        m = memory[own]                                     # [8, T_in, H]
        memPK_r = np.zeros((128, NPAIR, KC, 128), np.float16)
        for j in range(BL):
            memPK_r[(j % 2) * 64:(j % 2) * 64 + 64, j // 2] = (
                m[j].reshape(T_IN, KC, 128))
        isel_r = np.zeros((B, BL), np.float32)
        isel_r[r * BL + np.arange(BL), np.arange(BL)] = 1.0
        maps.append(dict(
            common,
            wz=wz_r.astype(bfnp),
            xp=xp_r.astype(bfnp),
            c0=np.ascontiguousarray(2.0 * enc_c[:, r * 128:(r + 1) * 128]),
            keysk=keysK_r.astype(bfnp),
            mempk=memPK_r,
            isel=isel_r,
            fcw=np.ascontiguousarray(
                fc_w[:, r * VS:(r + 1) * VS]).reshape(KC, 128, VS).astype(bfnp),
            fcbr=np.ascontiguousarray(
                np.broadcast_to(fc_b[r * VS:(r + 1) * VS], (128, VS)), np.float32),
        ))
    return maps


def kernel(**inputs):
    if "nc" not in _CACHE:
        _CACHE["nc"] = _build()
    nc = _CACHE["nc"]
    maps = _prep_inputs(inputs)
    res = run_bass_kernel_spmd(nc, maps, list(range(NC)))
    global LAST_RESULT
    LAST_RESULT = res
    out = np.concatenate([res.results[r]["out"] for r in range(NC)], axis=2)
    return out


LAST_RESULT = None


# revision 15
# speedup vs baseline: 1.7134x; 1.1069x over previous
"""Trainium2 Bass kernel for the Luong-attention LSTM decoder (nn_Decoder).

8-core strategy (v2, sharded recurrence + interleaved epilogue):
- Host folds Wa into the recurrence: z = [H*, ctx] @ wz + xp with
  wz = [0.5*(Wa_top@Wk_a + lstm_r); Wa_bot@Wk_a], xp precomputed (+bias,
  t=0 correction). State H* = 2*h2 (tanh-only gates, no ACT table
  switches); consumers pre-scaled by 0.5 on host.
- z matmul + gates column-sharded 8-way: core r computes gate columns
  [i_r|f_r|o_r|g_r] (128 each), keeps its c-state slice. AllGather #1
  reassembles transposed H*; AllGather #2 reassembles ctx. Single-DMA
  gathers.
- Attention batch-sharded (8 samples/core): cross-scores via 8 matmuls +
  matmul diagonal-select (SPMD-uniform via per-core one-hot isel).
- Epilogue (attn2 = [H*,ctx]@wa_eff, logits vocab-sharded 4000/core) is
  chunked and pumped into the AG-wait windows of later steps, keeping the
  PE HAM-warm and removing the serial tail. Pair history kept in SBUF.
"""
import sys

sys.path.insert(0, "/opt/trn_rl_repo")

from collections import deque

import numpy as np
import concourse.bass as bass
import concourse.tile as tile
from concourse import bacc, mybir
from concourse.bass_utils import run_bass_kernel_spmd

B, T_IN, T_DEC = 64, 64, 47
V, E, H = 32000, 512, 1024
NC = 8
BL = B // NC
VS = V // NC
KC = H // 128
NPAIR = BL // 2
GS = VS // 8
NPAIRS_T = (T_DEC + 1) // 2
F32 = mybir.dt.float32
F16 = mybir.dt.float16
BF16 = mybir.dt.bfloat16
TANH = mybir.ActivationFunctionType.Tanh
EXP = mybir.ActivationFunctionType.Exp
MULT = mybir.AluOpType.mult
ADD = mybir.AluOpType.add

_CACHE = {}


def _build():
    nc = bacc.Bacc(None, target_bir_lowering=False)

    wz_d = nc.dram_tensor("wz", [16, 128, 512], BF16, kind="ExternalInput")
    xp_d = nc.dram_tensor("xp", [T_DEC, B, 512], BF16, kind="ExternalInput")
    c0_d = nc.dram_tensor("c0", [B, 128], F32, kind="ExternalInput")
    keysk_d = nc.dram_tensor("keysk", [128, KC, BL, T_IN], BF16, kind="ExternalInput")
    mempk_d = nc.dram_tensor("mempk", [128, NPAIR, KC, 128], F16, kind="ExternalInput")
    isel_d = nc.dram_tensor("isel", [B, BL], F32, kind="ExternalInput")
    idf_d = nc.dram_tensor("idf", [64, 64], F32, kind="ExternalInput")
    wa_d = nc.dram_tensor("wa", [16, 128, H], BF16, kind="ExternalInput")
    fcw_d = nc.dram_tensor("fcw", [KC, 128, VS], BF16, kind="ExternalInput")
    fcbr_d = nc.dram_tensor("fcbr", [128, VS], F32, kind="ExternalInput")
    idb_d = nc.dram_tensor("idb", [128, 128], BF16, kind="ExternalInput")
    out_d = nc.dram_tensor("out", [B, T_DEC, VS], F32, kind="ExternalOutput")

    ag1i = [nc.dram_tensor(f"ag1i{t}", [64, 128], BF16) for t in range(T_DEC)]
    ag1o = [
        nc.dram_tensor(f"ag1o{t}", [NC * 64, 128], BF16, addr_space="Shared")
        for t in range(T_DEC)
    ]
    ag2i = [nc.dram_tensor(f"ag2i{t}", [128, 64], BF16) for t in range(T_DEC)]
    ag2o = [
        nc.dram_tensor(f"ag2o{t}", [NC * 128, 64], BF16, addr_space="Shared")
        for t in range(T_DEC)
    ]
    rg = [list(range(NC))]

    with tile.TileContext(nc) as tc:
        with (
            tc.tile_pool(name="one", bufs=1) as one,
            tc.tile_pool(name="work", bufs=2) as work,
            tc.tile_pool(name="gat", bufs=2) as gat,
            tc.tile_pool(name="hist", bufs=3) as hist,
            tc.tile_pool(name="ework", bufs=2) as ework,
            tc.tile_pool(name="zps", bufs=1, space="PSUM") as zps,
            tc.tile_pool(name="tps", bufs=2, space="PSUM") as tps,
            tc.tile_pool(name="aps", bufs=1, space="PSUM") as aps,
            tc.tile_pool(name="eps", bufs=2, space="PSUM") as eps,
            tc.tile_pool(name="fps", bufs=1, space="PSUM") as fps,
        ):
            # ---------------- resident tiles ----------------
            wz = one.tile([128, 16, 512], BF16)
            nc.sync.dma_start(wz[:], wz_d.rearrange("k p n -> p k n"))
            keysK = one.tile([128, KC, BL, T_IN], BF16)
            nc.sync.dma_start(keysK[:], keysk_d[:])
            memPK = one.tile([128, NPAIR, KC, 128], F16)
            nc.sync.dma_start(memPK[:], mempk_d[:])
            isel = one.tile([B, BL], F32, tag="isel")
            nc.sync.dma_start(isel[:], isel_d[:])
            idf = one.tile([64, 64], F32, tag="idf")
            nc.sync.dma_start(idf[:], idf_d[:])
            cst = one.tile([B, 128], F32, tag="cst")
            nc.sync.dma_start(cst[:], c0_d[:])
            wa_sb = one.tile([128, 16, H], BF16)
            nc.sync.dma_start(wa_sb[:], wa_d.rearrange("k p h -> p k h"))
            fcw_sb = one.tile([128, KC, VS], BF16)
            nc.sync.dma_start(fcw_sb[:], fcw_d.rearrange("k p v -> p k v"))
            identb = one.tile([128, 128], BF16)
            nc.sync.dma_start(identb[:], idb_d[:])
            fcbR = one.tile([128, VS], F32)
            nc.sync.dma_start(fcbR[:], fcbr_d[:])

            h2T = one.tile([128, KC, 64], BF16, tag="h2T")
            nc.vector.memset(h2T[:], 0.0)
            ctxT = one.tile([128, NC, KC, BL], BF16, tag="ctxT")
            nc.vector.memset(ctxT[:], 0.0)
            ctxTz = one.tile([128, KC, NC, BL], BF16, tag="ctxTz")
            nc.vector.memset(ctxTz[:], 0.0)
            alignZ = one.tile([128, BL], F16, tag="alignZ")
            nc.vector.memset(alignZ[:], 0.0)
            ones64 = one.tile([B, 1], F32, tag="ones64")
            nc.vector.memset(ones64[:], 1.0)
            onesr = one.tile([1, B], F32, tag="onesr")
            nc.vector.memset(onesr[:], 1.0)

            # ------------- epilogue chunk machinery -------------
            epiq = deque()

            def pump(budget_us):
                spent = 0.0
                while epiq and spent < budget_us:
                    cost, go = epiq.popleft()
                    go()
                    spent += cost

            def make_pair_chunks(t0, nsteps, h2p, ctxp):
                M = 64 * nsteps
                a2sb = ework.tile([128, H], BF16, tag="a2sb",
                                  name=f"a2sb{t0}")
                a2t = ework.tile([128, KC, 128], BF16, tag="a2t",
                                 name=f"a2t{t0}")

                a2cs = {}

                def a2_chunk(n, half):
                    def go():
                        if n not in a2cs:
                            a2cs[n] = eps.tile([128, 512], F32, tag="a2c",
                                               name=f"a2c{t0}_{n}")
                        a2c = a2cs[n]
                        for k in range(half * 8, half * 8 + 8):
                            if k < KC:
                                lhs = h2p[:, k, 0:nsteps, :].rearrange(
                                    "p s b -> p (s b)")
                            else:
                                lhs = ctxp[:, k - KC, 0:nsteps, :].rearrange(
                                    "p s b -> p (s b)")
                            nc.tensor.matmul(
                                a2c[0:M, :], lhs,
                                wa_sb[:, k, n * 512:(n + 1) * 512],
                                start=(k == 0), stop=(k == 15),
                            )
                        if half == 1:
                            nc.scalar.copy(a2sb[0:M, n * 512:(n + 1) * 512],
                                           a2c[0:M, :])
                    return go

                def tr_chunk():
                    for c2 in range(KC):
                        tpe = tps.tile([128, 128], BF16, tag="tp",
                                       name=f"a2tp{t0}_{c2}")
                        nc.tensor.transpose(
                            tpe[:, 0:M], a2sb[0:M, c2 * 128:(c2 + 1) * 128],
                            identb[0:M, 0:M])
                        nc.scalar.copy(a2t[:, c2, 0:M], tpe[:, 0:M])

                def fc_chunk(g):
                    def go():
                        lg = fps.tile([128, GS], F32, tag="lg",
                                      name=f"lg{t0}_{g}")
                        for k in range(KC):
                            nc.tensor.matmul(
                                lg[0:M, :], a2t[:, k, 0:M],
                                fcw_sb[:, k, g * GS:(g + 1) * GS],
                                start=(k == 0), stop=(k == KC - 1),
                            )
                        lgs = ework.tile([128, GS], F32, tag="lgs",
                                         name=f"lgs{t0}_{g}")
                        nc.vector.scalar_tensor_tensor(
                            lgs[0:M, :], lg[0:M, :], 1.0,
                            fcbR[0:M, g * GS:(g + 1) * GS], MULT, ADD)
                        for i in range(nsteps):
                            nc.gpsimd.dma_start(
                                out_d[:, t0 + i, g * GS:(g + 1) * GS],
                                lgs[i * 64:(i + 1) * 64, :],
                            )
                    return go

                for n in range(2):
                    for half in range(2):
                        epiq.append((1.8, a2_chunk(n, half)))
                epiq.append((2.6, tr_chunk))
                for g in range(8):
                    epiq.append((1.8, fc_chunk(g)))

            # ================= decode loop =================
            h2p_cur = ctxp_cur = None
            for t in range(T_DEC):
                xpt = work.tile([B, 512], BF16, tag="xp")
                nc.sync.dma_start(xpt[:], xp_d[t])

                # z = [H*,ctx] @ wz  (k 0..7: H* tiles; 8..15: ctx tiles)
                zq = zps.tile([B, 512], F32, tag="zq")
                for k in range(16):
                    if k < KC:
                        lhs = h2T[:, k]
                    else:
                        lhs = ctxTz[:, k - KC].rearrange("p q j -> p (q j)")
                    nc.tensor.matmul(
                        zq[:], lhs, wz[:, k],
                        start=(k == 0), stop=(k == 15),
                    )
                pump(2.0)

                # gates: layout [i|f|o|g] x128; H* = 2h, D = 2c
                z2 = gat.tile([B, 512], F32, tag="z2")
                nc.vector.scalar_tensor_tensor(z2[:], zq[:], 1.0, xpt[:], MULT, ADD)
                tio = gat.tile([B, 384], F32, tag="tio")
                nc.scalar.activation(tio[:], z2[:, 0:384], TANH, scale=0.5)
                tg = gat.tile([B, 128], F32, tag="tg")
                nc.scalar.activation(tg[:], z2[:, 384:512], TANH)
                av = gat.tile([B, 128], F32, tag="av")
                nc.vector.scalar_tensor_tensor(av[:], tio[:, 128:256], 1.0, cst[:], ADD, MULT)
                bv = gat.tile([B, 128], F32, tag="bv")
                nc.vector.scalar_tensor_tensor(bv[:], tio[:, 0:128], 1.0, tg[:], ADD, MULT)
                nc.vector.scalar_tensor_tensor(cst[:], av[:], 0.5, bv[:], MULT, ADD)
                tc2 = gat.tile([B, 128], F32, tag="tc2")
                nc.scalar.activation(tc2[:], cst[:], TANH, scale=0.5)
                h2s = gat.tile([B, 128], BF16, tag="h2s")
                nc.vector.scalar_tensor_tensor(h2s[:], tio[:, 256:384], 1.0, tc2[:], ADD, MULT)

                # stage own H* slice [64, 128]; transpose happens in the
                # gather DMA (xbar)
                nc.sync.dma_start(ag1i[t][:], h2s[:])
                nc.gpsimd.collective_compute(
                    "AllGather", mybir.AluOpType.bypass,
                    replica_groups=rg,
                    ins=[ag1i[t][:]], outs=[ag1o[t][:]],
                )
                pump(4.0)
                nc.scalar.dma_start(
                    h2T[:].rearrange("p q b -> p (q b)"), ag1o[t][:],
                    transpose=True,
                )

                # ---- scores for own 8 samples via cross + diag-select ----
                crossP = aps.tile([64, 512], F32, tag="crossP")
                for c in range(KC):
                    nc.tensor.matmul(
                        crossP[:], h2T[:, c],
                        keysK[:, c].rearrange("p j t -> p (j t)"),
                        start=(c == 0), stop=(c == KC - 1),
                    )
                ecr = gat.tile([64, 512], F32, tag="ecr")
                nc.scalar.activation(ecr[:], crossP[:], EXP)
                combo = aps.tile([128, KC * BL + 3 * BL], F32, tag="combo")
                ctxPS = combo[:, 0:KC * BL].rearrange("p (c j) -> p c j", c=KC)
                smx = combo[0:64, KC * BL:]
                for j in range(BL):
                    nc.tensor.matmul(
                        smx[:, j:j + 1], ecr[:, j * 64:(j + 1) * 64],
                        isel[:, j:j + 1], start=True, stop=True,
                    )
                e8 = gat.tile([64, BL], F32, tag="e8")
                nc.vector.tensor_copy(e8[:], smx[:, 0:BL])
                nc.tensor.matmul(smx[0:1, BL:2 * BL], ones64[:], e8[:],
                                 start=True, stop=True)
                r18 = gat.tile([1, BL], F32, tag="r18")
                nc.vector.reciprocal(r18[:], smx[0:1, BL:2 * BL])
                nc.tensor.matmul(smx[:, 2 * BL:3 * BL], onesr[:], r18[:],
                                 start=True, stop=True)
                a8 = gat.tile([64, BL], F16, tag="a8")
                nc.vector.tensor_mul(a8[:], e8[:], smx[:, 2 * BL:3 * BL])
                # scatter: even own-samples -> upper half, odd -> lower half
                nc.vector.tensor_copy(
                    alignZ[0:64, :].rearrange("p (pr two) -> p pr two", two=2)[:, :, 0],
                    a8[:].rearrange("p (pr two) -> p pr two", two=2)[:, :, 0],
                )
                nc.vector.tensor_copy(
                    alignZ[64:128, :].rearrange("p (pr two) -> p pr two", two=2)[:, :, 1],
                    a8[:].rearrange("p (pr two) -> p pr two", two=2)[:, :, 1],
                )

                # ---- ctx (own batches, pair-packed block-diag) ----
                for pr in range(NPAIR):
                    for c in range(KC):
                        nc.tensor.matmul(
                            ctxPS[:, c, 2 * pr:2 * pr + 2],
                            memPK[:, pr, c, :],
                            alignZ[:, 2 * pr:2 * pr + 2],
                            start=True, stop=True,
                        )
                ctxo = work.tile([128, KC, BL], BF16, tag="ctxo")
                nc.scalar.copy(ctxo[:], ctxPS[:])
                nc.sync.dma_start(ag2i[t][:], ctxo[:].rearrange("p c j -> p (c j)"))
                nc.gpsimd.collective_compute(
                    "AllGather", mybir.AluOpType.bypass,
                    replica_groups=rg,
                    ins=[ag2i[t][:]], outs=[ag2o[t][:]],
                )
                pump(10.0)
                nc.scalar.dma_start(
                    ctxT[:],
                    ag2o[t].rearrange("(q p) (c j) -> p q c j", q=NC, c=KC),
                )
                nc.vector.tensor_copy(
                    ctxTz[:], ctxT[:].rearrange("p q c j -> p c q j"))

                # ---- pair history (SBUF) + epilogue enqueue ----
                if t % 2 == 0:
                    h2p_cur = hist.tile([128, KC, 2, 64], BF16, tag="h2p",
                                        name=f"h2p{t}")
                    ctxp_cur = hist.tile([128, KC, 2, 64], BF16, tag="ctxp",
                                         name=f"ctxp{t}")
                slot = t % 2
                nc.vector.tensor_copy(h2p_cur[:, :, slot, :], h2T[:])
                nc.vector.tensor_copy(
                    ctxp_cur[:, :, slot, :],
                    ctxTz[:].rearrange("p c q j -> p c (q j)"))
                if t % 2 == 1:
                    make_pair_chunks(t - 1, 2, h2p_cur, ctxp_cur)
                elif t == T_DEC - 1:
                    make_pair_chunks(t, 1, h2p_cur, ctxp_cur)

            # drain remaining epilogue work
            pump(1e9)

    nc.finalize()
    return nc


def _prep_inputs(inputs):
    bfnp = mybir.dt.np(BF16)
    f32 = lambda x: np.asarray(x, dtype=np.float32)
    tokens = np.asarray(inputs["tokens"])
    memory = f32(inputs["memory"])
    enc_h = f32(inputs["enc_h"])
    enc_c = f32(inputs["enc_c"])
    emb = f32(inputs["emb"])
    Wm = f32(inputs["Wm"])
    Wa = f32(inputs["Wa"])
    lstm_k = f32(inputs["lstm_k"])
    lstm_r = f32(inputs["lstm_r"])
    lstm_b = f32(inputs["lstm_b"])
    fc_w = f32(inputs["fc_w"])
    fc_b = f32(inputs["fc_b"])

    Wk_x = lstm_k[:E]
    Wk_a = lstm_k[E:]
    Rp = Wa[:H] @ Wk_a + lstm_r
    Cp = Wa[H:] @ Wk_a
    wz_full = np.concatenate([0.5 * Rp, Cp], axis=0)       # [2048, 4096]
    xs = emb[tokens]                                        # [B, T, E]
    xpb = (xs @ Wk_x + lstm_b).transpose(1, 0, 2).copy()    # [T, B, 4096]
    xpb[0] += enc_h @ lstm_r                                # t=0 folding fix
    keys = memory @ (0.5 * Wm)                              # [B, T_in, H]
    wa_eff = np.concatenate([0.5 * Wa[:H], Wa[H:]], axis=0)

    common = dict(
        idf=np.eye(64, dtype=np.float32),
        idb=np.eye(128, dtype=np.float32).astype(bfnp),
        wa=np.ascontiguousarray(wa_eff.reshape(16, 128, H)).astype(bfnp),
    )
    maps = []
    for r in range(NC):
        cols = np.concatenate([
            np.arange(r * 128, r * 128 + 128),
            H + np.arange(r * 128, r * 128 + 128),
            3 * H + np.arange(r * 128, r * 128 + 128),
            2 * H + np.arange(r * 128, r * 128 + 128),
        ])
        wz_r = np.ascontiguousarray(wz_full[:, cols]).reshape(16, 128, 512)
        xp_r = np.ascontiguousarray(xpb[:, :, cols])
        own = slice(r * BL, (r + 1) * BL)
        kk = keys[own].transpose(2, 0, 1)                   # [H, 8, T_in]
        keysK_r = np.ascontiguousarray(
            kk.reshape(KC, 128, BL, T_IN).transpose(1, 0, 2, 3))
        m = memory[own]                                     # [8, T_in, H]
        memPK_r = np.zeros((128, NPAIR, KC, 128), np.float16)
        for j in range(BL):
            memPK_r[(j % 2) * 64:(j % 2) * 64 + 64, j // 2] = (
                m[j].reshape(T_IN, KC, 128))
        isel_r = np.zeros((B, BL), np.float32)
        isel_r[r * BL + np.arange(BL), np.arange(BL)] = 1.0
        maps.append(dict(
            common,
            wz=wz_r.astype(bfnp),
            xp=xp_r.astype(bfnp),
            c0=np.ascontiguousarray(2.0 * enc_c[:, r * 128:(r + 1) * 128]),
            keysk=keysK_r.astype(bfnp),
            mempk=memPK_r,
            isel=isel_r,
            fcw=np.ascontiguousarray(
                fc_w[:, r * VS:(r + 1) * VS]).reshape(KC, 128, VS).astype(bfnp),
            fcbr=np.ascontiguousarray(
                np.broadcast_to(fc_b[r * VS:(r + 1) * VS], (128, VS)), np.float32),
        ))
    return maps


def kernel(**inputs):
    if "nc" not in _CACHE:
        _CACHE["nc"] = _build()
    nc = _CACHE["nc"]
    maps = _prep_inputs(inputs)
    res = run_bass_kernel_spmd(nc, maps, list(range(NC)))
    global LAST_RESULT
    LAST_RESULT = res
    out = np.concatenate([res.results[r]["out"] for r in range(NC)], axis=2)
    return out


LAST_RESULT = None
